# revision 3
# baseline (speedup 1.0000x reference)
"""Trainium2 Bass kernel for Exaone4-style GQA attention block (T=2048, HID=4096,
H=32 q-heads, HK=8 kv-heads, D=128, sliding window 1023, QK-RMSNorm + NeoX RoPE).

Sharding: tensor-parallel over heads across 8 NeuronCores. Core m owns q-heads
[4m, 4m+4) and kv-head m (GQA group-aligned), plus the matching o_proj column
slice; per-core partial outputs are summed on the host (the all-reduce).

Device layout notes:
 - qkv projection is computed transposed ([feature, t]) so attention works in
   the S^T = K^T.T @ Q^T layout; softmax sums over the partition axis are done
   with ones-vector matmuls on the PE, and PV consumes exp(S^T) directly.
 - RMSNorm scale and RoPE are fused via host-precomputed [128, T] cos/sin
   tables (norm weights + 1/sqrt(D) folded in); the partition-half rotation
   for RoPE uses SBUF->SBUF DMA.
 - All large matmuls use bf16 operands with fp32 PSUM accumulation.
 - Emission order per t-block: attention(tb) -> qkv matmuls(tb+1) ->
   o_proj(tb-1) -> qkv tail(tb+1), with attention's PV/rowsum matmuls lagged
   LAG blocks behind their QK so the PE never stalls on the mask/exp chain.
"""

import sys

import numpy as np

if "/opt/trn_rl_repo" not in sys.path:
    sys.path.insert(0, "/opt/trn_rl_repo")

import ml_dtypes

BF16 = ml_dtypes.bfloat16

HID = 4096
H = 32
HK = 8
D = 128
WIN = 1023
THETA = 1000000.0
EPS = 1e-6
SCALE = D ** -0.5
M = 8            # cores
QH = H // M      # q heads per core (4)
NJ = QH + 2      # j-blocks in qkv^T output (4 q + 1 k + 1 v)
TB = 512         # t free-dim block
NEG = -1.0e30
LAG = 3          # attention PV/rowsum lag behind QK (covers mask+exp latency)

_PROG_CACHE = {}


def _build_program(T):
    """Build the (single-core SPMD) Bass program for sequence length T."""
    from contextlib import ExitStack

    import concourse.bass as bass  # noqa: F401
    import concourse.tile as tile
    from concourse import bacc, mybir
    from concourse.masks import make_identity

    f32 = mybir.dt.float32
    bf = mybir.dt.bfloat16

    NT = T // TB          # number of t blocks
    NC = HID // 128       # contraction chunks
    NOB = HID // 128      # output row blocks

    nc = bacc.Bacc(
        "TRN2",
        target_bir_lowering=False,
        debug=False,
        enable_asserts=False,
        num_devices=M,
    )

    # x pre-tiled on host: block (tb, cq) = [128, 4*TB], 4 c-chunks interleaved
    # per partition row (4KB contiguous per partition per DMA)
    xT_h = nc.dram_tensor(
        "xT", [(T // TB) * (HID // 512) * 128, 4 * TB], bf, kind="ExternalInput"
    )
    wq_h = nc.dram_tensor("wqkvT", [HID, NJ * 128], bf, kind="ExternalInput")
    wo_h = nc.dram_tensor("woT", [QH * 128, HID], bf, kind="ExternalInput")
    cwq_h = nc.dram_tensor("cwq", [128, T], bf, kind="ExternalInput")
    swq_h = nc.dram_tensor("swq", [128, T], bf, kind="ExternalInput")
    cwk_h = nc.dram_tensor("cwk", [128, T], bf, kind="ExternalInput")
    swk_h = nc.dram_tensor("swk", [128, T], bf, kind="ExternalInput")
    maskd_h = nc.dram_tensor("maskd", [128, 128], f32, kind="ExternalInput")
    maskw_h = nc.dram_tensor("maskw", [128, 128], f32, kind="ExternalInput")
    # out pre-tiled: block (tb, obp) = [128, 2*TB] (ob pairs interleaved per row)
    outT_h = nc.dram_tensor(
        "outT", [(T // TB) * (HID // 256) * 128, 2 * TB], bf, kind="ExternalOutput"
    )

    xTr = xT_h.ap().rearrange("(b p) u -> b p u", p=128)
    wqr = wq_h.ap().rearrange("(c p) j -> p c j", p=128)
    wor = wo_h.ap().rearrange("(jc p) o -> p jc o", p=128)
    outr = outT_h.ap().rearrange("(b p) u -> b p u", p=128)

    mult = mybir.AluOpType.mult
    add = mybir.AluOpType.add
    Exp = mybir.ActivationFunctionType.Exp
    Sqrt = mybir.ActivationFunctionType.Sqrt

    with tile.TileContext(nc) as tc, ExitStack() as ctx:
        singles = ctx.enter_context(tc.tile_pool(name="singles", bufs=1))
        persist = ctx.enter_context(tc.tile_pool(name="persist", bufs=1))
        xpool = ctx.enter_context(tc.tile_pool(name="xpool", bufs=4))
        stpool = ctx.enter_context(tc.tile_pool(name="stpool", bufs=1))
        ropep = ctx.enter_context(tc.tile_pool(name="ropep", bufs=2))
        espool = ctx.enter_context(tc.tile_pool(name="espool", bufs=5))
        outp = ctx.enter_context(tc.tile_pool(name="outp", bufs=2))
        smallp = ctx.enter_context(tc.tile_pool(name="smallp", bufs=2))
        # PSUM: every tile is <= one bank; a single tag with 8 rotating slots
        # covers all 8 banks and lets phases overlap freely.
        psum = ctx.enter_context(tc.tile_pool(name="psum", bufs=8, space="PSUM"))
        drp = ctx.enter_context(tc.tile_pool(name="drp", bufs=4, space="DRAM"))

        def bcast_row(src_row, tag):
            """Broadcast a [1, TB] sbuf row to a [128, TB] sbuf tile.

            SBUF sources cannot have zero partition step in a DMA, so bounce
            through a DRAM scratch row and broadcast-read it back."""
            drs = drp.tile([1, TB], f32, name=f"drs_{tag}", tag=f"dr_{tag}")
            nc.gpsimd.dma_start(drs, src_row)
            dst = ropep.tile([128, TB], f32, name=f"bc_{tag}", tag=tag)
            nc.gpsimd.dma_start(dst, drs.to_broadcast([128, TB]))
            return dst

        # ---- resident constants (cheap ones only; big DMAs staged below) ----
        maskd_sb = singles.tile([128, 128], f32)
        nc.sync.dma_start(maskd_sb, maskd_h.ap())
        maskw_sb = singles.tile([128, 128], f32)
        nc.sync.dma_start(maskw_sb, maskw_h.ap())
        ident = singles.tile([128, 128], bf)
        make_identity(nc, ident)
        ones_bf = singles.tile([128, 1], bf)
        nc.vector.memset(ones_bf, 1.0)
        eps_sb = singles.tile([128, 1], f32)
        nc.vector.memset(eps_sb, EPS)

        w_sb = singles.tile([128, NC, NJ * 128], bf)
        cwq_sb = singles.tile([128, T], bf)
        swq_sb = singles.tile([128, T], bf)
        cwk_sb = singles.tile([128, T], bf)
        swk_sb = singles.tile([128, T], bf)
        wo_sb = singles.tile([128, QH, HID], bf)

        # ---- persistent activations ---------------------------------------
        qT = persist.tile([128, QH, T], bf)     # rope'd+normed q^T
        kT = persist.tile([128, T], bf)         # rope'd+normed k^T
        Vt = persist.tile([128, T // 128, 128], bf)  # v in [s, d] layout

        stages = {}

        def phase_a_mm(tb, startup=False):
            """qkv projection matmuls for t block tb (single x pass, 6 banks).

            On startup, interleave the w chunk DMAs with the first x chunk
            DMAs on the sync queue so the PE starts ~5us in instead of
            waiting for the full 6.3MB weight load."""
            stage = stpool.tile(
                [128, NJ, TB], bf, tag="stage", bufs=2, name=f"stage_{tb}"
            )
            ps_all = [
                psum.tile([128, TB], f32, name=f"psqkv_{tb}_{j}", tag="bank")
                for j in range(NJ)
            ]
            for cq in range(NC // 4):
                if startup and cq % 2 == 0 and cq // 2 < 4:
                    k8 = cq // 2
                    nc.sync.dma_start(
                        w_sb[:, 8 * k8 : 8 * (k8 + 1), :], wqr[:, 8 * k8 : 8 * (k8 + 1), :]
                    )
                xc = xpool.tile([128, 4, TB], bf, tag="xc", name=f"xc_{tb}_{cq}")
                nc.sync.dma_start(
                    xc,
                    xTr[tb * (NC // 4) + cq].rearrange("p (ci u) -> p ci u", u=TB),
                )
                for ci in range(4):
                    c = 4 * cq + ci
                    for j in range(NJ):
                        nc.tensor.matmul(
                            ps_all[j],
                            lhsT=w_sb[:, c, j * 128 : (j + 1) * 128],
                            rhs=xc[:, ci, :],
                            start=(c == 0),
                            stop=(c == NC - 1),
                        )
            # PSUM -> SBUF casts split across vector/scalar engines
            for j in range(NJ):
                if j % 2 == 0:
                    nc.vector.tensor_copy(stage[:, j], ps_all[j])
                else:
                    nc.scalar.copy(stage[:, j], ps_all[j])
            stages[tb] = stage

        def phase_a_tail(tb):
            """v transpose + per-head RMSNorm scale + RoPE for t block tb."""
            t0 = tb * TB
            ts_ = slice(t0, t0 + TB)
            stage = stages.pop(tb)

            # v: transpose [d, t] -> [s, d] blocks via PE
            for u in range(TB // 128):
                ps_t = psum.tile([128, 128], bf, name=f"pst_{tb}_{u}", tag="bank")
                nc.tensor.transpose(ps_t, stage[:, QH + 1, u * 128 : (u + 1) * 128], ident)
                nc.scalar.copy(Vt[:, tb * (TB // 128) + u, :], ps_t)

            # rms scale: 1/sqrt(mean(x^2) + eps) per j-block via ones-matmul
            scls = []
            for j in range(QH + 1):
                sq = stpool.tile([128, TB], bf, tag="sq", bufs=2, name=f"sq_{tb}_{j}")
                nc.vector.tensor_tensor(sq, stage[:, j], stage[:, j], mult)
                ps_ss = psum.tile([1, TB], f32, name=f"psss_{tb}_{j}", tag="bank")
                nc.tensor.matmul(ps_ss, lhsT=ones_bf, rhs=sq, start=True, stop=True)
                rms = smallp.tile([1, TB], f32, tag="rms", name=f"rms_{tb}_{j}")
                nc.scalar.activation(rms, ps_ss, Sqrt, bias=eps_sb[0:1, :], scale=1.0 / D)
                scl = smallp.tile([1, TB], f32, tag="scl", name=f"scl_{tb}_{j}")
                nc.vector.reciprocal_approx_fast(scl, rms)
                scls.append(scl)

            for j in range(QH + 1):
                sclb = bcast_row(scls[j], "sclb")
                qn = ropep.tile([128, TB], f32, tag="qn", name=f"qn_{tb}_{j}")
                nc.vector.tensor_tensor(qn, stage[:, j], sclb, mult)
                qrot = ropep.tile([128, TB], f32, tag="qrot", name=f"qrot_{tb}_{j}")
                nc.gpsimd.dma_start(qrot[0:64, :], qn[64:128, :])
                nc.gpsimd.dma_start(qrot[64:128, :], qn[0:64, :])
                cw = cwq_sb if j < QH else cwk_sb
                sw = swq_sb if j < QH else swk_sb
                b_t = ropep.tile([128, TB], f32, tag="b_t", name=f"bt_{tb}_{j}")
                nc.vector.tensor_tensor(b_t, qrot, sw[:, ts_], mult)
                nc.vector.tensor_tensor(qn, qn, cw[:, ts_], mult)
                dest = qT[:, j, ts_] if j < QH else kT[:, ts_]
                nc.vector.tensor_tensor(dest, qn, b_t, add)

        attnTs = {}

        def phase_b(tb):
            """attention for t block tb (attnT kept for phase_c).

            Per head: QK/mask/exp runs LAG o-blocks ahead of PV/rowsum so the
            PE stream never waits on the vector/scalar exp chain."""
            t0 = tb * TB
            # o = sb - 4*tb; o=0 (full col range) goes FIRST so the
            # start=True PV/rowsum matmuls cover the whole bank; later
            # partial-range matmuls accumulate onto uniformly-written bytes
            # (CoreSim requires this; matches HW has_written semantics).
            obs = [0] + [o for o in range(-8, 4) if o != 0 and 4 * tb + o >= 0]
            attnT = outp.tile([128, QH, TB], bf, tag="attnT", name=f"attnT_{tb}")
            attnTs[tb] = attnT
            nob = len(obs)
            for h in range(QH):
                pv = psum.tile([128, TB], f32, name=f"pspv_{tb}_{h}", tag="bank")
                rs = psum.tile([1, TB], f32, name=f"psr_{tb}_{h}", tag="bank")
                ess = {}
                rngs = {}

                def emit_pv(oi):
                    o = obs[oi]
                    c0, c1 = rngs[oi]
                    first = oi == 0
                    last = oi == nob - 1
                    nc.tensor.matmul(
                        pv[:, c0:c1],
                        lhsT=Vt[:, 4 * tb + o, :],
                        rhs=ess[oi][:, c0:c1],
                        start=first,
                        stop=last,
                        skip_group_check=True,
                    )
                    nc.tensor.matmul(
                        rs[:, c0:c1],
                        lhsT=ones_bf,
                        rhs=ess[oi][:, c0:c1],
                        start=first,
                        stop=last,
                        skip_group_check=True,
                    )
                    del ess[oi]

                for oi, o in enumerate(obs):
                    sb = 4 * tb + o
                    if o >= 0:
                        c0, c1 = 128 * o, TB
                    elif o >= -4:
                        c0, c1 = 0, TB
                    else:
                        c0, c1 = 0, 128 * (o + 9)
                    rngs[oi] = (c0, c1)
                    ps_s = psum.tile(
                        [128, TB], f32, name=f"pss_{tb}_{h}_{oi}", tag="bank"
                    )
                    nc.tensor.matmul(
                        ps_s[:, c0:c1],
                        lhsT=kT[:, sb * 128 : (sb + 1) * 128],
                        rhs=qT[:, h, t0 + c0 : t0 + c1],
                        start=True,
                        stop=True,
                    )
                    if o >= 0:  # causal strip at cols [128o, 128o+128)
                        u0 = 128 * o
                        nc.vector.tensor_tensor(
                            ps_s[:, u0 : u0 + 128], ps_s[:, u0 : u0 + 128],
                            maskd_sb, add,
                        )
                    elif o <= -5:  # window strip
                        u0 = 128 * (o + 8)
                        nc.vector.tensor_tensor(
                            ps_s[:, u0 : u0 + 128], ps_s[:, u0 : u0 + 128],
                            maskw_sb, add,
                        )
                    es = espool.tile(
                        [128, TB], bf, tag="es", name=f"es_{tb}_{h}_{oi}"
                    )
                    nc.scalar.activation(es[:, c0:c1], ps_s[:, c0:c1], Exp)
                    ess[oi] = es
                    if oi >= LAG:
                        emit_pv(oi - LAG)
                for oi in range(max(0, nob - LAG), nob):
                    emit_pv(oi)

                # normalize: attnT[:, h] = pv * (1/rowsum) broadcast
                rsum = smallp.tile([1, TB], f32, tag="rsum", name=f"rsum_{tb}_{h}")
                nc.vector.tensor_copy(rsum, rs)
                nc.vector.reciprocal_approx_fast(rsum, rsum)
                rb = bcast_row(rsum, "sclb")
                nc.vector.tensor_tensor(attnT[:, h, :], pv, rb, mult)

        def phase_c(tb):
            """o_proj partial for t block tb (store ob pairs as one DMA)."""
            attnT = attnTs.pop(tb)
            for obp in range(NOB // 2):
                o_st = outp.tile(
                    [128, 2, TB], bf, tag="o_st", bufs=3, name=f"ost_{tb}_{obp}"
                )
                for oi in range(2):
                    ob = 2 * obp + oi
                    ps_o = psum.tile([128, TB], f32, name=f"pso_{tb}_{ob}", tag="bank")
                    for jc in range(QH):
                        nc.tensor.matmul(
                            ps_o,
                            lhsT=wo_sb[:, jc, ob * 128 : (ob + 1) * 128],
                            rhs=attnT[:, jc, :],
                            start=(jc == 0),
                            stop=(jc == QH - 1),
                        )
                    if oi == 0:
                        nc.vector.tensor_copy(o_st[:, oi, :], ps_o)
                    else:
                        nc.scalar.copy(o_st[:, oi, :], ps_o)
                nc.sync.dma_start(
                    outr[tb * (NOB // 2) + obp].rearrange("p (oi u) -> p oi u", u=TB),
                    o_st,
                )

        # Software pipeline. Emission order per iteration:
        #   B(tb): attention (its mask/exp vector+scalar ops queue first)
        #   A_mm(tb+1): next block's qkv matmuls (dense PE filler)
        #   C(tb-1): previous block's o_proj (dense PE filler; its PSUM
        #            copies overlap A's rope chain)
        #   A_tail(tb+1): v transpose + rms + rope (PE bits tiny; vector
        #            chain runs under C/B of the next iteration)
        phase_a_mm(0, startup=True)
        # big constant DMAs queued after the first x/w interleave: rope
        # tables (needed by A_tail(0) ~45us in) then wo (needed by C(0))
        nc.sync.dma_start(cwq_sb, cwq_h.ap())
        nc.sync.dma_start(swq_sb, swq_h.ap())
        nc.sync.dma_start(cwk_sb, cwk_h.ap())
        nc.sync.dma_start(swk_sb, swk_h.ap())
        nc.sync.dma_start(wo_sb, wor)
        phase_a_tail(0)
        for tb in range(NT):
            phase_b(tb)
            if tb + 1 < NT:
                phase_a_mm(tb + 1)
            if tb >= 1:
                phase_c(tb - 1)
            if tb + 1 < NT:
                phase_a_tail(tb + 1)
        phase_c(NT - 1)

    nc.compile()
    return nc


def _get_program(T):
    if T not in _PROG_CACHE:
        _PROG_CACHE[T] = _build_program(T)
    return _PROG_CACHE[T]


def _host_prep(positions, hidden_states, wqkv, wo, q_norm_w, k_norm_w):
    """Build the 8 per-core input maps (host-side sharding + table prep)."""
    T = hidden_states.shape[0]
    pos = np.asarray(positions).astype(np.float64)
    hs = np.asarray(hidden_states, dtype=np.float32)
    wqkv = np.asarray(wqkv, dtype=np.float32)
    wo = np.asarray(wo, dtype=np.float32)
    qw = np.asarray(q_norm_w, dtype=np.float64)
    kw = np.asarray(k_norm_w, dtype=np.float64)

    half = D // 2
    inv_freq = 1.0 / (THETA ** (np.arange(0, D, 2, dtype=np.float64) / D))  # [64]
    th = pos[:, None] * inv_freq[None, :]          # [T, 64]
    cos = np.cos(th).T                             # [64, T] float64
    sin = np.sin(th).T

    def tables(w, scale):
        cw = np.empty((D, T), np.float64)
        sw = np.empty((D, T), np.float64)
        cw[:half] = cos * (w[:half, None] * scale)
        cw[half:] = cos * (w[half:, None] * scale)
        # out[d<64] = qn[d]*w[d]*cos - qn[d+64]*w[d+64]*sin  (rot reads qn[d+64])
        sw[:half] = -sin * (w[half:, None] * scale)
        # out[d>=64] = qn[d]*w[d]*cos + qn[d-64]*w[d-64]*sin
        sw[half:] = sin * (w[:half, None] * scale)
        return cw.astype(BF16), sw.astype(BF16)

    cwq, swq = tables(qw, SCALE)
    cwk, swk = tables(kw, 1.0)

    si = np.arange(128)[:, None]
    ui = np.arange(128)[None, :]
    maskd = np.where(ui >= si, 0.0, NEG).astype(np.float32)
    maskw = np.where(ui < si, 0.0, NEG).astype(np.float32)

    # tiled layout: block (tb, cq) = [128, 4*TB]; row p holds c-chunks
    # 4cq..4cq+3 back to back (4KB contiguous per partition)
    NTb, NCq = T // TB, HID // 512
    xT = np.ascontiguousarray(
        hs.T.reshape(NCq, 4, 128, NTb, TB)
        .transpose(3, 0, 2, 1, 4)
        .reshape(NTb * NCq * 128, 4 * TB)
    ).astype(BF16)

    in_maps = []
    for m in range(M):
        wq_m = wqkv[m * QH * D : (m + 1) * QH * D]            # [512, HID]
        wk_m = wqkv[H * D + m * D : H * D + (m + 1) * D]      # [128, HID]
        wv_m = wqkv[(H + HK) * D + m * D : (H + HK) * D + (m + 1) * D]
        wqkvT_m = np.ascontiguousarray(
            np.concatenate([wq_m, wk_m, wv_m], axis=0).T
        ).astype(BF16)                                        # [HID, 768]
        woT_m = np.ascontiguousarray(
            wo[:, m * QH * D : (m + 1) * QH * D].T
        ).astype(BF16)                                        # [512, HID]
        in_maps.append(
            {
                "xT": xT,
                "wqkvT": wqkvT_m,
                "woT": woT_m,
                "cwq": cwq,
                "swq": swq,
                "cwk": cwk,
                "swk": swk,
                "maskd": maskd,
                "maskw": maskw,
            }
        )
    return in_maps


def _run(in_maps, T, trace=False):
    from concourse import bass_utils

    nc = _get_program(T)
    res = bass_utils.run_bass_kernel_spmd(
        nc, in_maps, core_ids=list(range(M)), trace=trace
    )
    return res


def kernel(positions, hidden_states, wqkv, wo, q_norm_w, k_norm_w, _trace=False):
    T = hidden_states.shape[0]
    in_maps = _host_prep(positions, hidden_states, wqkv, wo, q_norm_w, k_norm_w)
    res = _run(in_maps, T, trace=_trace)
    NTb, NOBp = T // TB, HID // 256
    acc = np.zeros((NTb, NOBp, 128, 2, TB), np.float64)
    for r in res.results:
        acc += r["outT"].astype(np.float64).reshape(NTb, NOBp, 128, 2, TB)
    # untile: out[t, o] with o = (2*obp + oi)*128 + p, t = tb*TB + u
    out = np.ascontiguousarray(
        acc.transpose(0, 4, 1, 3, 2).reshape(T, HID)
    ).astype(np.float32)
    kernel._last_results = res
    return out


# revision 5
# speedup vs baseline: 1.0432x; 1.0432x over previous
"""Trainium2 Bass kernel for Exaone4-style GQA attention block (T=2048, HID=4096,
H=32 q-heads, HK=8 kv-heads, D=128, sliding window 1023, QK-RMSNorm + NeoX RoPE).

Sharding: tensor-parallel over heads across 8 NeuronCores. Core m owns q-heads
[4m, 4m+4) and kv-head m (GQA group-aligned), plus the matching o_proj column
slice; per-core partial outputs are summed on the host (the all-reduce).

Device layout notes:
 - qkv projection is computed transposed ([feature, t]) so attention works in
   the S^T = K^T.T @ Q^T layout; softmax sums over the partition axis are done
   with ones-vector matmuls on the PE, and PV consumes exp(S^T) directly.
 - RMSNorm scale and RoPE are fused via host-precomputed [128, T] cos/sin
   tables (norm weights + 1/sqrt(D) folded in); the partition-half rotation
   for RoPE uses SBUF->SBUF DMA.
 - All large matmuls use bf16 operands with fp32 PSUM accumulation.
 - Schedule per iteration: attention(tb) with o_proj(tb-1) matmuls injected
   every few units (o_proj has no exp dependency, so it fills the PE while
   the scalar engine streams exps), then the rms+rope chain for tb+1 (vector/
   gpsimd), then qkv matmuls for tb+2 whose dense PE phase hides that chain.
"""

import sys

import numpy as np

if "/opt/trn_rl_repo" not in sys.path:
    sys.path.insert(0, "/opt/trn_rl_repo")

import ml_dtypes

BF16 = ml_dtypes.bfloat16

HID = 4096
H = 32
HK = 8
D = 128
WIN = 1023
THETA = 1000000.0
EPS = 1e-6
SCALE = D ** -0.5
M = 8            # cores
QH = H // M      # q heads per core (4)
NJ = QH + 2      # j-blocks in qkv^T output (4 q + 1 k + 1 v)
TB = 512         # t free-dim block
NEG = -1.0e30
LAG = 3          # attention PV/rowsum lag behind QK (covers mask+exp latency)

_PROG_CACHE = {}


def _build_program(T):
    """Build the (single-core SPMD) Bass program for sequence length T."""
    from contextlib import ExitStack

    import concourse.bass as bass  # noqa: F401
    import concourse.tile as tile
    from concourse import bacc, mybir
    from concourse.masks import make_identity

    f32 = mybir.dt.float32
    bf = mybir.dt.bfloat16

    NT = T // TB          # number of t blocks
    NC = HID // 128       # contraction chunks
    NOB = HID // 128      # output row blocks

    nc = bacc.Bacc(
        "TRN2",
        target_bir_lowering=False,
        debug=False,
        enable_asserts=False,
        num_devices=M,
    )

    # x pre-tiled on host: block (tb, cq) = [128, 4*TB], 4 c-chunks interleaved
    # per partition row (4KB contiguous per partition per DMA)
    xT_h = nc.dram_tensor(
        "xT", [(T // TB) * (HID // 512) * 128, 4 * TB], bf, kind="ExternalInput"
    )
    wq_h = nc.dram_tensor("wqkvT", [HID, NJ * 128], bf, kind="ExternalInput")
    wo_h = nc.dram_tensor("woT", [QH * 128, HID], bf, kind="ExternalInput")
    cwq_h = nc.dram_tensor("cwq", [128, T], bf, kind="ExternalInput")
    swq_h = nc.dram_tensor("swq", [128, T], bf, kind="ExternalInput")
    cwk_h = nc.dram_tensor("cwk", [128, T], bf, kind="ExternalInput")
    swk_h = nc.dram_tensor("swk", [128, T], bf, kind="ExternalInput")
    maskd_h = nc.dram_tensor("maskd", [128, 128], f32, kind="ExternalInput")
    maskw_h = nc.dram_tensor("maskw", [128, 128], f32, kind="ExternalInput")
    # out pre-tiled: block (tb, obp) = [128, 2*TB] (ob pairs interleaved per row)
    outT_h = nc.dram_tensor(
        "outT", [(T // TB) * (HID // 256) * 128, 2 * TB], bf, kind="ExternalOutput"
    )

    xTr = xT_h.ap().rearrange("(b p) u -> b p u", p=128)
    wqr = wq_h.ap().rearrange("(c p) j -> p c j", p=128)
    wor = wo_h.ap().rearrange("(jc p) o -> p jc o", p=128)
    outr = outT_h.ap().rearrange("(b p) u -> b p u", p=128)

    mult = mybir.AluOpType.mult
    add = mybir.AluOpType.add
    Exp = mybir.ActivationFunctionType.Exp
    Sqrt = mybir.ActivationFunctionType.Sqrt

    with tile.TileContext(nc) as tc, ExitStack() as ctx:
        singles = ctx.enter_context(tc.tile_pool(name="singles", bufs=1))
        persist = ctx.enter_context(tc.tile_pool(name="persist", bufs=1))
        xpool = ctx.enter_context(tc.tile_pool(name="xpool", bufs=4))
        stpool = ctx.enter_context(tc.tile_pool(name="stpool", bufs=1))
        ropep = ctx.enter_context(tc.tile_pool(name="ropep", bufs=2))
        espool = ctx.enter_context(tc.tile_pool(name="espool", bufs=5))
        outp = ctx.enter_context(tc.tile_pool(name="outp", bufs=2))
        smallp = ctx.enter_context(tc.tile_pool(name="smallp", bufs=2))
        # PSUM: every tile is <= one bank; a single tag with 8 rotating slots
        # covers all 8 banks and lets phases overlap freely.
        psum = ctx.enter_context(tc.tile_pool(name="psum", bufs=8, space="PSUM"))
        drp = ctx.enter_context(tc.tile_pool(name="drp", bufs=4, space="DRAM"))

        def bcast_row(src_row, tag):
            """Broadcast a [1, TB] sbuf row to a [128, TB] sbuf tile.

            SBUF sources cannot have zero partition step in a DMA, so bounce
            through a DRAM scratch row and broadcast-read it back."""
            drs = drp.tile([1, TB], f32, name=f"drs_{tag}", tag=f"dr_{tag}")
            nc.gpsimd.dma_start(drs, src_row)
            dst = ropep.tile([128, TB], f32, name=f"bc_{tag}", tag=tag)
            nc.gpsimd.dma_start(dst, drs.to_broadcast([128, TB]))
            return dst

        # ---- cheap resident constants (no DMA) ----------------------------
        ident = singles.tile([128, 128], bf)
        make_identity(nc, ident)
        ones_bf = singles.tile([128, 1], bf)
        nc.vector.memset(ones_bf, 1.0)
        ones_row = singles.tile([1, 128], bf)
        nc.vector.memset(ones_row, 1.0)
        eps_sb = singles.tile([128, 1], f32)
        nc.vector.memset(eps_sb, EPS)

        maskd_sb = singles.tile([128, 128], f32)
        maskw_sb = singles.tile([128, 128], f32)
        w_sb = singles.tile([128, NC, NJ * 128], bf)
        cwq_sb = singles.tile([128, T], bf)
        swq_sb = singles.tile([128, T], bf)
        cwk_sb = singles.tile([128, T], bf)
        swk_sb = singles.tile([128, T], bf)
        wo_sb = singles.tile([128, QH, HID], bf)

        # ---- persistent activations ---------------------------------------
        qT = persist.tile([128, QH, T], bf)     # rope'd+normed q^T
        kT = persist.tile([128, T], bf)         # rope'd+normed k^T
        Vt = persist.tile([128, T // 128, 128], bf)  # v in [s, d] layout

        stages = {}

        def phase_a_mm(tb, startup=False):
            """qkv projection matmuls for t block tb (single x pass, 6 banks).

            On startup, interleave the w chunk DMAs with the x chunk DMAs on
            the sync queue so the PE starts ~4us in instead of waiting for
            the full 6.3MB weight load."""
            stage = stpool.tile(
                [128, NJ, TB], bf, tag="stage", bufs=2, name=f"stage_{tb}"
            )
            ps_all = [
                psum.tile([128, TB], f32, name=f"psqkv_{tb}_{j}", tag="bank")
                for j in range(NJ)
            ]
            for cq in range(NC // 4):
                if startup:
                    nc.sync.dma_start(
                        w_sb[:, 4 * cq : 4 * (cq + 1), :],
                        wqr[:, 4 * cq : 4 * (cq + 1), :],
                    )
                xc = xpool.tile([128, 4, TB], bf, tag="xc", name=f"xc_{tb}_{cq}")
                nc.sync.dma_start(
                    xc,
                    xTr[tb * (NC // 4) + cq].rearrange("p (ci u) -> p ci u", u=TB),
                )
                for ci in range(4):
                    c = 4 * cq + ci
                    for j in range(NJ):
                        nc.tensor.matmul(
                            ps_all[j],
                            lhsT=w_sb[:, c, j * 128 : (j + 1) * 128],
                            rhs=xc[:, ci, :],
                            start=(c == 0),
                            stop=(c == NC - 1),
                        )
            # PSUM -> SBUF casts split across vector/scalar engines
            for j in range(NJ):
                if j % 2 == 0:
                    nc.vector.tensor_copy(stage[:, j], ps_all[j])
                else:
                    nc.scalar.copy(stage[:, j], ps_all[j])
            stages[tb] = stage

        def phase_a_tail(tb):
            """v transpose + per-head RMSNorm scale + RoPE for t block tb.

            The chain is split across engines (sq/recip/qn/cw on vector,
            rot/b_t/dest on gpsimd, sqrt on scalar, broadcast via a 1-row PE
            matmul) so its serial latency stays under the qkv matmul phase
            that is emitted right after it."""
            t0 = tb * TB
            ts_ = slice(t0, t0 + TB)
            stage = stages.pop(tb)

            # v: transpose [d, t] -> [s, d] blocks via PE; copies grouped on
            # scalar right after the (Copy-table) stage copies
            for u in range(TB // 128):
                ps_t = psum.tile([128, 128], bf, name=f"pst_{tb}_{u}", tag="bank")
                nc.tensor.transpose(ps_t, stage[:, QH + 1, u * 128 : (u + 1) * 128], ident)
                nc.scalar.copy(Vt[:, tb * (TB // 128) + u, :], ps_t)

            # rms scale: 1/sqrt(mean(x^2) + eps) per j-block via ones-matmul
            rbs = []
            for j in range(QH + 1):
                sq = stpool.tile([128, TB], bf, tag="sq", bufs=2, name=f"sq_{tb}_{j}")
                nc.vector.tensor_tensor(sq, stage[:, j], stage[:, j], mult)
                ps_ss = psum.tile([1, TB], f32, name=f"psss_{tb}_{j}", tag="bank")
                nc.tensor.matmul(ps_ss, lhsT=ones_bf, rhs=sq, start=True, stop=True)
                rms = smallp.tile([1, TB], f32, tag="rms", name=f"rms_{tb}_{j}")
                nc.scalar.activation(rms, ps_ss, Sqrt, bias=eps_sb[0:1, :], scale=1.0 / D)
                scl = smallp.tile([1, TB], f32, tag="scl", name=f"scl_{tb}_{j}")
                nc.vector.reciprocal_approx_fast(scl, rms)
                scl_bf = smallp.tile([1, TB], bf, tag="sclb", name=f"sclb_{tb}_{j}")
                nc.vector.tensor_copy(scl_bf, scl)
                # broadcast 1/rms to all 128 partitions via a 1-contraction
                # matmul (no DRAM bounce, no DMA latency on this chain)
                rb = psum.tile([128, TB], f32, name=f"rb_{tb}_{j}", tag="bank")
                nc.tensor.matmul(rb, lhsT=ones_row, rhs=scl_bf, start=True, stop=True)
                rbs.append(rb)

            for j in range(QH + 1):
                qn = ropep.tile([128, TB], f32, tag="qn", name=f"qn_{tb}_{j}")
                nc.vector.tensor_tensor(qn, stage[:, j], rbs[j], mult)
                qrot = ropep.tile([128, TB], f32, tag="qrot", name=f"qrot_{tb}_{j}")
                nc.gpsimd.dma_start(qrot[0:64, :], qn[64:128, :])
                nc.gpsimd.dma_start(qrot[64:128, :], qn[0:64, :])
                cw = cwq_sb if j < QH else cwk_sb
                sw = swq_sb if j < QH else swk_sb
                b_t = ropep.tile([128, TB], f32, tag="b_t", name=f"bt_{tb}_{j}")
                nc.gpsimd.tensor_tensor(b_t, qrot, sw[:, ts_], mult)
                nc.vector.tensor_tensor(qn, qn, cw[:, ts_], mult)
                dest = qT[:, j, ts_] if j < QH else kT[:, ts_]
                nc.gpsimd.tensor_tensor(dest, qn, b_t, add)

        attnTs = {}

        def phase_bc(tb, ctb):
            """attention for t block tb with o_proj for block ctb interleaved.

            Per head: QK/mask/exp runs LAG o-blocks ahead of PV/rowsum so the
            PE stream never waits on the vector/scalar exp chain. o_proj
            ob-pairs (pure PE work, no exp dependency) are injected every few
            units to keep the PE ahead of the scalar engine's exp rate."""
            t0 = tb * TB
            # o = sb - 4*tb; o=0 (full col range) goes FIRST so the
            # start=True PV/rowsum matmuls cover the whole bank; later
            # partial-range matmuls accumulate onto uniformly-written bytes
            # (CoreSim requires this; matches HW has_written semantics).
            obs = [0] + [o for o in range(-8, 4) if o != 0 and 4 * tb + o >= 0]
            attnT = outp.tile([128, QH, TB], bf, tag="attnT", name=f"attnT_{tb}")
            attnTs[tb] = attnT
            nob = len(obs)

            c_units = list(range(NOB // 2)) if ctb is not None else []
            cattnT = attnTs.pop(ctb) if ctb is not None else None
            n_units = QH * nob
            stride = max(1, (n_units + len(c_units) - 1) // max(1, len(c_units))) \
                if c_units else n_units + 1
            unit = 0

            def emit_c_unit(obp):
                o_st = outp.tile(
                    [128, 2, TB], bf, tag="o_st", bufs=3, name=f"ost_{ctb}_{obp}"
                )
                for oi2 in range(2):
                    ob = 2 * obp + oi2
                    ps_o = psum.tile(
                        [128, TB], f32, name=f"pso_{ctb}_{ob}", tag="bank"
                    )
                    for jc in range(QH):
                        nc.tensor.matmul(
                            ps_o,
                            lhsT=wo_sb[:, jc, ob * 128 : (ob + 1) * 128],
                            rhs=cattnT[:, jc, :],
                            start=(jc == 0),
                            stop=(jc == QH - 1),
                        )
                    nc.vector.tensor_copy(o_st[:, oi2, :], ps_o)
                nc.sync.dma_start(
                    outr[ctb * (NOB // 2) + obp].rearrange(
                        "p (oi u) -> p oi u", u=TB
                    ),
                    o_st,
                )

            def tick():
                nonlocal unit
                unit += 1
                if c_units and unit % stride == 0:
                    emit_c_unit(c_units.pop(0))

            for h in range(QH):
                pv = psum.tile([128, TB], f32, name=f"pspv_{tb}_{h}", tag="bank")
                rs = psum.tile([1, TB], f32, name=f"psr_{tb}_{h}", tag="bank")
                ess = {}
                rngs = {}

                def emit_pv(oi):
                    o = obs[oi]
                    c0, c1 = rngs[oi]
                    first = oi == 0
                    last = oi == nob - 1
                    nc.tensor.matmul(
                        pv[:, c0:c1],
                        lhsT=Vt[:, 4 * tb + o, :],
                        rhs=ess[oi][:, c0:c1],
                        start=first,
                        stop=last,
                        skip_group_check=True,
                    )
                    nc.tensor.matmul(
                        rs[:, c0:c1],
                        lhsT=ones_bf,
                        rhs=ess[oi][:, c0:c1],
                        start=first,
                        stop=last,
                        skip_group_check=True,
                    )
                    del ess[oi]

                for oi, o in enumerate(obs):
                    sb = 4 * tb + o
                    if o >= 0:
                        c0, c1 = 128 * o, TB
                    elif o >= -4:
                        c0, c1 = 0, TB
                    else:
                        c0, c1 = 0, 128 * (o + 9)
                    rngs[oi] = (c0, c1)
                    ps_s = psum.tile(
                        [128, TB], f32, name=f"pss_{tb}_{h}_{oi}", tag="bank"
                    )
                    nc.tensor.matmul(
                        ps_s[:, c0:c1],
                        lhsT=kT[:, sb * 128 : (sb + 1) * 128],
                        rhs=qT[:, h, t0 + c0 : t0 + c1],
                        start=True,
                        stop=True,
                    )
                    if o >= 0:  # causal strip at cols [128o, 128o+128)
                        u0 = 128 * o
                        nc.vector.tensor_tensor(
                            ps_s[:, u0 : u0 + 128], ps_s[:, u0 : u0 + 128],
                            maskd_sb, add,
                        )
                    elif o <= -5:  # window strip
                        u0 = 128 * (o + 8)
                        nc.vector.tensor_tensor(
                            ps_s[:, u0 : u0 + 128], ps_s[:, u0 : u0 + 128],
                            maskw_sb, add,
                        )
                    es = espool.tile(
                        [128, TB], bf, tag="es", name=f"es_{tb}_{h}_{oi}"
                    )
                    nc.scalar.activation(es[:, c0:c1], ps_s[:, c0:c1], Exp)
                    ess[oi] = es
                    if oi >= LAG:
                        emit_pv(oi - LAG)
                    tick()
                for oi in range(max(0, nob - LAG), nob):
                    emit_pv(oi)

                # normalize: attnT[:, h] = pv * (1/rowsum) broadcast
                rsum = smallp.tile([1, TB], f32, tag="rsum", name=f"rsum_{tb}_{h}")
                nc.vector.tensor_copy(rsum, rs)
                nc.vector.reciprocal_approx_fast(rsum, rsum)
                rb = bcast_row(rsum, "sclb")
                nc.vector.tensor_tensor(attnT[:, h, :], pv, rb, mult)

            # leftover o_proj units (ragged stride)
            while c_units:
                emit_c_unit(c_units.pop(0))

        def phase_c(tb):
            """o_proj for t block tb standalone (epilogue)."""
            attnT = attnTs.pop(tb)
            for obp in range(NOB // 2):
                o_st = outp.tile(
                    [128, 2, TB], bf, tag="o_st", bufs=3, name=f"ost_{tb}_{obp}"
                )
                for oi in range(2):
                    ob = 2 * obp + oi
                    ps_o = psum.tile([128, TB], f32, name=f"pso_{tb}_{ob}", tag="bank")
                    for jc in range(QH):
                        nc.tensor.matmul(
                            ps_o,
                            lhsT=wo_sb[:, jc, ob * 128 : (ob + 1) * 128],
                            rhs=attnT[:, jc, :],
                            start=(jc == 0),
                            stop=(jc == QH - 1),
                        )
                    if oi == 0:
                        nc.vector.tensor_copy(o_st[:, oi, :], ps_o)
                    else:
                        nc.scalar.copy(o_st[:, oi, :], ps_o)
                nc.sync.dma_start(
                    outr[tb * (NOB // 2) + obp].rearrange("p (oi u) -> p oi u", u=TB),
                    o_st,
                )

        # Software pipeline (see module docstring). Prologue overlaps the
        # first rope chain with the second qkv matmul phase.
        phase_a_mm(0, startup=True)
        nc.sync.dma_start(cwq_sb, cwq_h.ap())
        nc.sync.dma_start(swq_sb, swq_h.ap())
        nc.sync.dma_start(cwk_sb, cwk_h.ap())
        nc.sync.dma_start(swk_sb, swk_h.ap())
        nc.sync.dma_start(maskd_sb, maskd_h.ap())
        nc.sync.dma_start(maskw_sb, maskw_h.ap())
        phase_a_mm(1)
        nc.sync.dma_start(wo_sb, wor)
        phase_a_tail(0)
        for tb in range(NT):
            phase_bc(tb, tb - 1 if tb >= 1 else None)
            if tb + 1 < NT:
                phase_a_tail(tb + 1)
            if tb + 2 < NT:
                phase_a_mm(tb + 2)
        phase_c(NT - 1)

    nc.compile()
    return nc


def _get_program(T):
    if T not in _PROG_CACHE:
        _PROG_CACHE[T] = _build_program(T)
    return _PROG_CACHE[T]


def _host_prep(positions, hidden_states, wqkv, wo, q_norm_w, k_norm_w):
    """Build the 8 per-core input maps (host-side sharding + table prep)."""
    T = hidden_states.shape[0]
    pos = np.asarray(positions).astype(np.float64)
    hs = np.asarray(hidden_states, dtype=np.float32)
    wqkv = np.asarray(wqkv, dtype=np.float32)
    wo = np.asarray(wo, dtype=np.float32)
    qw = np.asarray(q_norm_w, dtype=np.float64)
    kw = np.asarray(k_norm_w, dtype=np.float64)

    half = D // 2
    inv_freq = 1.0 / (THETA ** (np.arange(0, D, 2, dtype=np.float64) / D))  # [64]
    th = pos[:, None] * inv_freq[None, :]          # [T, 64]
    cos = np.cos(th).T                             # [64, T] float64
    sin = np.sin(th).T

    def tables(w, scale):
        cw = np.empty((D, T), np.float64)
        sw = np.empty((D, T), np.float64)
        cw[:half] = cos * (w[:half, None] * scale)
        cw[half:] = cos * (w[half:, None] * scale)
        # out[d<64] = qn[d]*w[d]*cos - qn[d+64]*w[d+64]*sin  (rot reads qn[d+64])
        sw[:half] = -sin * (w[half:, None] * scale)
        # out[d>=64] = qn[d]*w[d]*cos + qn[d-64]*w[d-64]*sin
        sw[half:] = sin * (w[:half, None] * scale)
        return cw.astype(BF16), sw.astype(BF16)

    cwq, swq = tables(qw, SCALE)
    cwk, swk = tables(kw, 1.0)

    si = np.arange(128)[:, None]
    ui = np.arange(128)[None, :]
    maskd = np.where(ui >= si, 0.0, NEG).astype(np.float32)
    maskw = np.where(ui < si, 0.0, NEG).astype(np.float32)

    # tiled layout: block (tb, cq) = [128, 4*TB]; row p holds c-chunks
    # 4cq..4cq+3 back to back (4KB contiguous per partition)
    NTb, NCq = T // TB, HID // 512
    xT = np.ascontiguousarray(
        hs.T.reshape(NCq, 4, 128, NTb, TB)
        .transpose(3, 0, 2, 1, 4)
        .reshape(NTb * NCq * 128, 4 * TB)
    ).astype(BF16)

    in_maps = []
    for m in range(M):
        wq_m = wqkv[m * QH * D : (m + 1) * QH * D]            # [512, HID]
        wk_m = wqkv[H * D + m * D : H * D + (m + 1) * D]      # [128, HID]
        wv_m = wqkv[(H + HK) * D + m * D : (H + HK) * D + (m + 1) * D]
        wqkvT_m = np.ascontiguousarray(
            np.concatenate([wq_m, wk_m, wv_m], axis=0).T
        ).astype(BF16)                                        # [HID, 768]
        woT_m = np.ascontiguousarray(
            wo[:, m * QH * D : (m + 1) * QH * D].T
        ).astype(BF16)                                        # [512, HID]
        in_maps.append(
            {
                "xT": xT,
                "wqkvT": wqkvT_m,
                "woT": woT_m,
                "cwq": cwq,
                "swq": swq,
                "cwk": cwk,
                "swk": swk,
                "maskd": maskd,
                "maskw": maskw,
            }
        )
    return in_maps


def _run(in_maps, T, trace=False):
    from concourse import bass_utils

    nc = _get_program(T)
    res = bass_utils.run_bass_kernel_spmd(
        nc, in_maps, core_ids=list(range(M)), trace=trace
    )
    return res


def kernel(positions, hidden_states, wqkv, wo, q_norm_w, k_norm_w, _trace=False):
    T = hidden_states.shape[0]
    in_maps = _host_prep(positions, hidden_states, wqkv, wo, q_norm_w, k_norm_w)
    res = _run(in_maps, T, trace=_trace)
    NTb, NOBp = T // TB, HID // 256
    acc = np.zeros((NTb, NOBp, 128, 2, TB), np.float64)
    for r in res.results:
        acc += r["outT"].astype(np.float64).reshape(NTb, NOBp, 128, 2, TB)
    # untile: out[t, o] with o = (2*obp + oi)*128 + p, t = tb*TB + u
    out = np.ascontiguousarray(
        acc.transpose(0, 4, 1, 3, 2).reshape(T, HID)
    ).astype(np.float32)
    kernel._last_results = res
    return out


# revision 8
# speedup vs baseline: 1.0745x; 1.0300x over previous
"""Trainium2 Bass kernel for Exaone4-style GQA attention block (T=2048, HID=4096,
H=32 q-heads, HK=8 kv-heads, D=128, sliding window 1023, QK-RMSNorm + NeoX RoPE).

Sharding: tensor-parallel over heads across 8 NeuronCores. Core m owns q-heads
[4m, 4m+4) and kv-head m (GQA group-aligned), plus the matching o_proj column
slice; per-core partial outputs are summed on the host (the all-reduce).

Device layout notes:
 - qkv projection is computed transposed ([feature, t]) so attention works in
   the S^T = K^T.T @ Q^T layout; softmax sums over the partition axis are done
   with ones-vector matmuls on the PE, and PV consumes exp(S^T) directly.
 - RMSNorm scale and RoPE are fused via host-precomputed [128, T] cos/sin
   tables (norm weights + 1/sqrt(D) folded in); the partition-half rotation
   for RoPE uses SBUF->SBUF DMA.
 - All large matmuls use bf16 operands with fp32 PSUM accumulation.
 - Schedule per iteration: attention(tb) with o_proj(tb-1) matmuls injected
   every few units (o_proj has no exp dependency, so it fills the PE while
   the scalar engine streams exps), then the rms+rope chain for tb+1 (vector/
   gpsimd), then qkv matmuls for tb+2 whose dense PE phase hides that chain.
"""

import sys

import numpy as np

if "/opt/trn_rl_repo" not in sys.path:
    sys.path.insert(0, "/opt/trn_rl_repo")

import ml_dtypes

BF16 = ml_dtypes.bfloat16

HID = 4096
H = 32
HK = 8
D = 128
WIN = 1023
THETA = 1000000.0
EPS = 1e-6
SCALE = D ** -0.5
M = 8            # cores
QH = H // M      # q heads per core (4)
NJ = QH + 2      # j-blocks in qkv^T output (4 q + 1 k + 1 v)
TB = 512         # t free-dim block
NEG = -1.0e30
LAG = 3          # attention PV/rowsum lag behind QK (covers mask+exp latency)

_PROG_CACHE = {}


def _build_program(T):
    """Build the (single-core SPMD) Bass program for sequence length T."""
    from contextlib import ExitStack

    import concourse.bass as bass  # noqa: F401
    import concourse.tile as tile
    from concourse import bacc, mybir
    from concourse.masks import make_identity

    f32 = mybir.dt.float32
    bf = mybir.dt.bfloat16

    NT = T // TB          # number of t blocks
    NC = HID // 128       # contraction chunks
    NOB = HID // 128      # output row blocks

    nc = bacc.Bacc(
        "TRN2",
        target_bir_lowering=False,
        debug=False,
        enable_asserts=False,
        num_devices=M,
    )

    # x pre-tiled on host: block (tb, cq) = [128, 4*TB], 4 c-chunks interleaved
    # per partition row (4KB contiguous per partition per DMA)
    xT_h = nc.dram_tensor(
        "xT", [(T // TB) * (HID // 512) * 128, 4 * TB], bf, kind="ExternalInput"
    )
    wq_h = nc.dram_tensor("wqkvT", [HID, NJ * 128], bf, kind="ExternalInput")
    wo_h = nc.dram_tensor("woT", [QH * 128, HID], bf, kind="ExternalInput")
    cwq_h = nc.dram_tensor("cwq", [128, T], bf, kind="ExternalInput")
    swq_h = nc.dram_tensor("swq", [128, T], bf, kind="ExternalInput")
    cwk_h = nc.dram_tensor("cwk", [128, T], bf, kind="ExternalInput")
    swk_h = nc.dram_tensor("swk", [128, T], bf, kind="ExternalInput")
    maskd_h = nc.dram_tensor("maskd", [128, 128], f32, kind="ExternalInput")
    maskw_h = nc.dram_tensor("maskw", [128, 128], f32, kind="ExternalInput")
    # out pre-tiled: block (tb, obp) = [128, 2*TB] (ob pairs interleaved per row)
    outT_h = nc.dram_tensor(
        "outT", [(T // TB) * (HID // 256) * 128, 2 * TB], bf, kind="ExternalOutput"
    )

    xTr = xT_h.ap().rearrange("(b p) u -> b p u", p=128)
    wqr = wq_h.ap().rearrange("(c p) j -> p c j", p=128)
    wor = wo_h.ap().rearrange("(jc p) o -> p jc o", p=128)
    outr = outT_h.ap().rearrange("(b p) u -> b p u", p=128)

    mult = mybir.AluOpType.mult
    add = mybir.AluOpType.add
    Exp = mybir.ActivationFunctionType.Exp
    Sqrt = mybir.ActivationFunctionType.Sqrt

    with tile.TileContext(nc) as tc, ExitStack() as ctx:
        singles = ctx.enter_context(tc.tile_pool(name="singles", bufs=1))
        persist = ctx.enter_context(tc.tile_pool(name="persist", bufs=1))
        xpool = ctx.enter_context(tc.tile_pool(name="xpool", bufs=4))
        stpool = ctx.enter_context(tc.tile_pool(name="stpool", bufs=1))
        ropep = ctx.enter_context(tc.tile_pool(name="ropep", bufs=2))
        espool = ctx.enter_context(tc.tile_pool(name="espool", bufs=5))
        outp = ctx.enter_context(tc.tile_pool(name="outp", bufs=2))
        smallp = ctx.enter_context(tc.tile_pool(name="smallp", bufs=2))
        # PSUM: every tile is <= one bank; a single tag with 8 rotating slots
        # covers all 8 banks and lets phases overlap freely.
        psum = ctx.enter_context(tc.tile_pool(name="psum", bufs=8, space="PSUM"))
        drp = ctx.enter_context(tc.tile_pool(name="drp", bufs=4, space="DRAM"))

        def bcast_row(src_row, tag):
            """Broadcast a [1, TB] sbuf row to a [128, TB] sbuf tile.

            SBUF sources cannot have zero partition step in a DMA, so bounce
            through a DRAM scratch row and broadcast-read it back."""
            drs = drp.tile([1, TB], f32, name=f"drs_{tag}", tag=f"dr_{tag}")
            nc.gpsimd.dma_start(drs, src_row)
            dst = ropep.tile([128, TB], f32, name=f"bc_{tag}", tag=tag)
            nc.gpsimd.dma_start(dst, drs.to_broadcast([128, TB]))
            return dst

        # ---- cheap resident constants (no DMA) ----------------------------
        ident = singles.tile([128, 128], bf)
        make_identity(nc, ident)
        ones_bf = singles.tile([128, 1], bf)
        nc.vector.memset(ones_bf, 1.0)
        ones_row = singles.tile([1, 128], bf)
        nc.vector.memset(ones_row, 1.0)
        eps_sb = singles.tile([128, 1], f32)
        nc.vector.memset(eps_sb, EPS)

        maskd_sb = singles.tile([128, 128], f32)
        maskw_sb = singles.tile([128, 128], f32)
        w_sb = singles.tile([128, NC, NJ * 128], bf)
        cwq_sb = singles.tile([128, T], bf)
        swq_sb = singles.tile([128, T], bf)
        cwk_sb = singles.tile([128, T], bf)
        swk_sb = singles.tile([128, T], bf)
        wo_sb = singles.tile([128, QH, HID], bf)

        # ---- persistent activations ---------------------------------------
        qT = persist.tile([128, QH, T], bf)     # rope'd+normed q^T
        kT = persist.tile([128, T], bf)         # rope'd+normed k^T
        Vt = persist.tile([128, T // 128, 128], bf)  # v in [s, d] layout

        stages = {}

        def phase_a_mm(tb, startup=False):
            """qkv projection matmuls for t block tb (single x pass, 6 banks).

            On startup, interleave the w chunk DMAs with the x chunk DMAs on
            the sync queue so the PE starts ~4us in instead of waiting for
            the full 6.3MB weight load."""
            stage = stpool.tile(
                [128, NJ, TB], bf, tag="stage", bufs=2, name=f"stage_{tb}"
            )
            ps_all = [
                psum.tile([128, TB], f32, name=f"psqkv_{tb}_{j}", tag="bank")
                for j in range(NJ)
            ]
            for cq in range(NC // 4):
                if startup:
                    nc.sync.dma_start(
                        w_sb[:, 4 * cq : 4 * cq + 2, :],
                        wqr[:, 4 * cq : 4 * cq + 2, :],
                    )
                xc = xpool.tile([128, 4, TB], bf, tag="xc", name=f"xc_{tb}_{cq}")
                nc.sync.dma_start(
                    xc,
                    xTr[tb * (NC // 4) + cq].rearrange("p (ci u) -> p ci u", u=TB),
                )
                if startup:
                    nc.sync.dma_start(
                        w_sb[:, 4 * cq + 2 : 4 * (cq + 1), :],
                        wqr[:, 4 * cq + 2 : 4 * (cq + 1), :],
                    )
                for ci in range(4):
                    c = 4 * cq + ci
                    for j in range(NJ):
                        nc.tensor.matmul(
                            ps_all[j],
                            lhsT=w_sb[:, c, j * 128 : (j + 1) * 128],
                            rhs=xc[:, ci, :],
                            start=(c == 0),
                            stop=(c == NC - 1),
                        )
            # PSUM -> SBUF casts on vector (keeps the scalar engine's
            # activation table parked on Exp/Sqrt)
            for j in range(NJ):
                nc.vector.tensor_copy(stage[:, j], ps_all[j])
            stages[tb] = stage

        def phase_a_tail(tb):
            """v transpose + per-head RMSNorm scale + RoPE for t block tb.

            The chain is split across engines (sq/recip/qn/cw on vector,
            rot/b_t/dest on gpsimd, sqrt on scalar, broadcast via a 1-row PE
            matmul) so its serial latency stays under the qkv matmul phase
            that is emitted right after it."""
            t0 = tb * TB
            ts_ = slice(t0, t0 + TB)
            stage = stages.pop(tb)

            # v: transpose [d, t] -> [s, d] blocks via PE
            for u in range(TB // 128):
                ps_t = psum.tile([128, 128], bf, name=f"pst_{tb}_{u}", tag="bank")
                nc.tensor.transpose(ps_t, stage[:, QH + 1, u * 128 : (u + 1) * 128], ident)
                nc.vector.tensor_copy(Vt[:, tb * (TB // 128) + u, :], ps_t)

            # rms scale: 1/sqrt(mean(x^2) + eps) per j-block via ones-matmul
            rbs = []
            for j in range(QH + 1):
                sq = stpool.tile([128, TB], bf, tag="sq", bufs=2, name=f"sq_{tb}_{j}")
                nc.vector.tensor_tensor(sq, stage[:, j], stage[:, j], mult)
                ps_ss = psum.tile([1, TB], f32, name=f"psss_{tb}_{j}", tag="bank")
                nc.tensor.matmul(ps_ss, lhsT=ones_bf, rhs=sq, start=True, stop=True)
                rms = smallp.tile([1, TB], f32, tag="rms", name=f"rms_{tb}_{j}")
                nc.scalar.activation(rms, ps_ss, Sqrt, bias=eps_sb[0:1, :], scale=1.0 / D)
                scl = smallp.tile([1, TB], f32, tag="scl", name=f"scl_{tb}_{j}")
                nc.vector.reciprocal_approx_fast(scl, rms)
                scl_bf = smallp.tile([1, TB], bf, tag="sclb", name=f"sclb_{tb}_{j}")
                nc.vector.tensor_copy(scl_bf, scl)
                # broadcast 1/rms to all 128 partitions via a 1-contraction
                # matmul (no DRAM bounce, no DMA latency on this chain)
                rb = psum.tile([128, TB], f32, name=f"rb_{tb}_{j}", tag="bank")
                nc.tensor.matmul(rb, lhsT=ones_row, rhs=scl_bf, start=True, stop=True)
                rbs.append(rb)

            for j in range(QH + 1):
                qn = ropep.tile([128, TB], f32, tag="qn", name=f"qn_{tb}_{j}")
                nc.vector.tensor_tensor(qn, stage[:, j], rbs[j], mult)
                qrot = ropep.tile([128, TB], f32, tag="qrot", name=f"qrot_{tb}_{j}")
                nc.gpsimd.dma_start(qrot[0:64, :], qn[64:128, :])
                nc.gpsimd.dma_start(qrot[64:128, :], qn[0:64, :])
                cw = cwq_sb if j < QH else cwk_sb
                sw = swq_sb if j < QH else swk_sb
                b_t = ropep.tile([128, TB], f32, tag="b_t", name=f"bt_{tb}_{j}")
                nc.gpsimd.tensor_tensor(b_t, qrot, sw[:, ts_], mult)
                nc.vector.tensor_tensor(qn, qn, cw[:, ts_], mult)
                dest = qT[:, j, ts_] if j < QH else kT[:, ts_]
                nc.gpsimd.tensor_tensor(dest, qn, b_t, add)

        attnTs = {}

        def phase_bc(tb, ctb):
            """attention for t block tb with o_proj for block ctb interleaved.

            Per head: QK/mask/exp runs LAG o-blocks ahead of PV/rowsum so the
            PE stream never waits on the vector/scalar exp chain. o_proj
            ob-pairs (pure PE work, no exp dependency) are injected every few
            units to keep the PE ahead of the scalar engine's exp rate."""
            t0 = tb * TB
            # o = sb - 4*tb; o=0 (full col range) goes FIRST so the
            # start=True PV/rowsum matmuls cover the whole bank; later
            # partial-range matmuls accumulate onto uniformly-written bytes
            # (CoreSim requires this; matches HW has_written semantics).
            obs = [0] + [o for o in range(-8, 4) if o != 0 and 4 * tb + o >= 0]
            attnT = outp.tile([128, QH, TB], bf, tag="attnT", name=f"attnT_{tb}")
            attnTs[tb] = attnT
            nob = len(obs)

            c_units = list(range(NOB // 2)) if ctb is not None else []
            cattnT = attnTs.pop(ctb) if ctb is not None else None
            n_units = QH * nob
            stride = max(1, (n_units + len(c_units) - 1) // max(1, len(c_units))) \
                if c_units else n_units + 1
            unit = 0

            def emit_c_unit(obp):
                o_st = outp.tile(
                    [128, 2, TB], bf, tag="o_st", bufs=3, name=f"ost_{ctb}_{obp}"
                )
                for oi2 in range(2):
                    ob = 2 * obp + oi2
                    ps_o = psum.tile(
                        [128, TB], f32, name=f"pso_{ctb}_{ob}", tag="bank"
                    )
                    for jc in range(QH):
                        nc.tensor.matmul(
                            ps_o,
                            lhsT=wo_sb[:, jc, ob * 128 : (ob + 1) * 128],
                            rhs=cattnT[:, jc, :],
                            start=(jc == 0),
                            stop=(jc == QH - 1),
                        )
                    nc.vector.tensor_copy(o_st[:, oi2, :], ps_o)
                nc.sync.dma_start(
                    outr[ctb * (NOB // 2) + obp].rearrange(
                        "p (oi u) -> p oi u", u=TB
                    ),
                    o_st,
                )

            def tick():
                nonlocal unit
                unit += 1
                if c_units and unit % stride == 0:
                    emit_c_unit(c_units.pop(0))

            for h in range(QH):
                pv = psum.tile([128, TB], f32, name=f"pspv_{tb}_{h}", tag="bank")
                rs = psum.tile([1, TB], f32, name=f"psr_{tb}_{h}", tag="bank")
                ess = {}
                rngs = {}

                def emit_pv(oi):
                    o = obs[oi]
                    c0, c1 = rngs[oi]
                    first = oi == 0
                    last = oi == nob - 1
                    nc.tensor.matmul(
                        pv[:, c0:c1],
                        lhsT=Vt[:, 4 * tb + o, :],
                        rhs=ess[oi][:, c0:c1],
                        start=first,
                        stop=last,
                        skip_group_check=True,
                    )
                    nc.tensor.matmul(
                        rs[:, c0:c1],
                        lhsT=ones_bf,
                        rhs=ess[oi][:, c0:c1],
                        start=first,
                        stop=last,
                        skip_group_check=True,
                    )
                    del ess[oi]

                for oi, o in enumerate(obs):
                    sb = 4 * tb + o
                    if o >= 0:
                        c0, c1 = 128 * o, TB
                    elif o >= -4:
                        c0, c1 = 0, TB
                    else:
                        c0, c1 = 0, 128 * (o + 9)
                    rngs[oi] = (c0, c1)
                    ps_s = psum.tile(
                        [128, TB], f32, name=f"pss_{tb}_{h}_{oi}", tag="bank"
                    )
                    nc.tensor.matmul(
                        ps_s[:, c0:c1],
                        lhsT=kT[:, sb * 128 : (sb + 1) * 128],
                        rhs=qT[:, h, t0 + c0 : t0 + c1],
                        start=True,
                        stop=True,
                    )
                    if o >= 0:  # causal strip at cols [128o, 128o+128)
                        u0 = 128 * o
                        nc.vector.tensor_tensor(
                            ps_s[:, u0 : u0 + 128], ps_s[:, u0 : u0 + 128],
                            maskd_sb, add,
                        )
                    elif o <= -5:  # window strip
                        u0 = 128 * (o + 8)
                        nc.vector.tensor_tensor(
                            ps_s[:, u0 : u0 + 128], ps_s[:, u0 : u0 + 128],
                            maskw_sb, add,
                        )
                    es = espool.tile(
                        [128, TB], bf, tag="es", name=f"es_{tb}_{h}_{oi}"
                    )
                    nc.scalar.activation(es[:, c0:c1], ps_s[:, c0:c1], Exp)
                    ess[oi] = es
                    if oi >= LAG:
                        emit_pv(oi - LAG)
                    tick()
                for oi in range(max(0, nob - LAG), nob):
                    emit_pv(oi)

                # normalize: attnT[:, h] = pv * (1/rowsum) broadcast
                rsum = smallp.tile([1, TB], f32, tag="rsum", name=f"rsum_{tb}_{h}")
                nc.vector.tensor_copy(rsum, rs)
                nc.vector.reciprocal_approx_fast(rsum, rsum)
                rb = bcast_row(rsum, "sclb")
                nc.vector.tensor_tensor(attnT[:, h, :], pv, rb, mult)

            # leftover o_proj units (ragged stride)
            while c_units:
                emit_c_unit(c_units.pop(0))

        def phase_c(tb):
            """o_proj for t block tb standalone (epilogue)."""
            attnT = attnTs.pop(tb)
            for obp in range(NOB // 2):
                o_st = outp.tile(
                    [128, 2, TB], bf, tag="o_st", bufs=3, name=f"ost_{tb}_{obp}"
                )
                for oi in range(2):
                    ob = 2 * obp + oi
                    ps_o = psum.tile([128, TB], f32, name=f"pso_{tb}_{ob}", tag="bank")
                    for jc in range(QH):
                        nc.tensor.matmul(
                            ps_o,
                            lhsT=wo_sb[:, jc, ob * 128 : (ob + 1) * 128],
                            rhs=attnT[:, jc, :],
                            start=(jc == 0),
                            stop=(jc == QH - 1),
                        )
                    if oi == 0:
                        nc.vector.tensor_copy(o_st[:, oi, :], ps_o)
                    else:
                        nc.scalar.copy(o_st[:, oi, :], ps_o)
                nc.sync.dma_start(
                    outr[tb * (NOB // 2) + obp].rearrange("p (oi u) -> p oi u", u=TB),
                    o_st,
                )

        # Software pipeline (see module docstring). Per iteration the vector
        # queue is [bc's masks/copies][A_tail chain][A_mm stage copies]: the
        # rope chain's engine work drains during bc's and A_mm's dense PE
        # phases, and its outputs are only consumed two iterations later.
        phase_a_mm(0, startup=True)
        nc.sync.dma_start(cwq_sb, cwq_h.ap())
        nc.sync.dma_start(swq_sb, swq_h.ap())
        nc.sync.dma_start(cwk_sb, cwk_h.ap())
        nc.sync.dma_start(swk_sb, swk_h.ap())
        phase_a_tail(0)
        phase_a_mm(1)
        nc.sync.dma_start(maskd_sb, maskd_h.ap())
        nc.sync.dma_start(maskw_sb, maskw_h.ap())
        nc.sync.dma_start(wo_sb, wor)
        phase_a_tail(1)
        phase_a_mm(2)
        for tb in range(NT):
            phase_bc(tb, tb - 1 if tb >= 1 else None)
            if tb + 2 < NT:
                phase_a_tail(tb + 2)
            if tb + 3 < NT:
                phase_a_mm(tb + 3)
        phase_c(NT - 1)

    nc.compile()
    return nc


def _get_program(T):
    if T not in _PROG_CACHE:
        _PROG_CACHE[T] = _build_program(T)
    return _PROG_CACHE[T]


def _host_prep(positions, hidden_states, wqkv, wo, q_norm_w, k_norm_w):
    """Build the 8 per-core input maps (host-side sharding + table prep)."""
    T = hidden_states.shape[0]
    pos = np.asarray(positions).astype(np.float64)
    hs = np.asarray(hidden_states, dtype=np.float32)
    wqkv = np.asarray(wqkv, dtype=np.float32)
    wo = np.asarray(wo, dtype=np.float32)
    qw = np.asarray(q_norm_w, dtype=np.float64)
    kw = np.asarray(k_norm_w, dtype=np.float64)

    half = D // 2
    inv_freq = 1.0 / (THETA ** (np.arange(0, D, 2, dtype=np.float64) / D))  # [64]
    th = pos[:, None] * inv_freq[None, :]          # [T, 64]
    cos = np.cos(th).T                             # [64, T] float64
    sin = np.sin(th).T

    def tables(w, scale):
        cw = np.empty((D, T), np.float64)
        sw = np.empty((D, T), np.float64)
        cw[:half] = cos * (w[:half, None] * scale)
        cw[half:] = cos * (w[half:, None] * scale)
        # out[d<64] = qn[d]*w[d]*cos - qn[d+64]*w[d+64]*sin  (rot reads qn[d+64])
        sw[:half] = -sin * (w[half:, None] * scale)
        # out[d>=64] = qn[d]*w[d]*cos + qn[d-64]*w[d-64]*sin
        sw[half:] = sin * (w[:half, None] * scale)
        return cw.astype(BF16), sw.astype(BF16)

    cwq, swq = tables(qw, SCALE)
    cwk, swk = tables(kw, 1.0)

    si = np.arange(128)[:, None]
    ui = np.arange(128)[None, :]
    maskd = np.where(ui >= si, 0.0, NEG).astype(np.float32)
    maskw = np.where(ui < si, 0.0, NEG).astype(np.float32)

    # tiled layout: block (tb, cq) = [128, 4*TB]; row p holds c-chunks
    # 4cq..4cq+3 back to back (4KB contiguous per partition)
    NTb, NCq = T // TB, HID // 512
    xT = np.ascontiguousarray(
        hs.T.reshape(NCq, 4, 128, NTb, TB)
        .transpose(3, 0, 2, 1, 4)
        .reshape(NTb * NCq * 128, 4 * TB)
    ).astype(BF16)

    in_maps = []
    for m in range(M):
        wq_m = wqkv[m * QH * D : (m + 1) * QH * D]            # [512, HID]
        wk_m = wqkv[H * D + m * D : H * D + (m + 1) * D]      # [128, HID]
        wv_m = wqkv[(H + HK) * D + m * D : (H + HK) * D + (m + 1) * D]
        wqkvT_m = np.ascontiguousarray(
            np.concatenate([wq_m, wk_m, wv_m], axis=0).T
        ).astype(BF16)                                        # [HID, 768]
        woT_m = np.ascontiguousarray(
            wo[:, m * QH * D : (m + 1) * QH * D].T
        ).astype(BF16)                                        # [512, HID]
        in_maps.append(
            {
                "xT": xT,
                "wqkvT": wqkvT_m,
                "woT": woT_m,
                "cwq": cwq,
                "swq": swq,
                "cwk": cwk,
                "swk": swk,
                "maskd": maskd,
                "maskw": maskw,
            }
        )
    return in_maps


def _run(in_maps, T, trace=False):
    from concourse import bass_utils

    nc = _get_program(T)
    res = bass_utils.run_bass_kernel_spmd(
        nc, in_maps, core_ids=list(range(M)), trace=trace
    )
    return res


def kernel(positions, hidden_states, wqkv, wo, q_norm_w, k_norm_w, _trace=False):
    T = hidden_states.shape[0]
    in_maps = _host_prep(positions, hidden_states, wqkv, wo, q_norm_w, k_norm_w)
    res = _run(in_maps, T, trace=_trace)
    NTb, NOBp = T // TB, HID // 256
    acc = np.zeros((NTb, NOBp, 128, 2, TB), np.float64)
    for r in res.results:
        acc += r["outT"].astype(np.float64).reshape(NTb, NOBp, 128, 2, TB)
    # untile: out[t, o] with o = (2*obp + oi)*128 + p, t = tb*TB + u
    out = np.ascontiguousarray(
        acc.transpose(0, 4, 1, 3, 2).reshape(T, HID)
    ).astype(np.float32)
    kernel._last_results = res
    return out


# revision 14
# speedup vs baseline: 1.2065x; 1.1228x over previous
"""Trainium2 Bass kernel for Exaone4-style GQA attention block (T=2048, HID=4096,
H=32 q-heads, HK=8 kv-heads, D=128, sliding window 1023, QK-RMSNorm + NeoX RoPE).

Sharding: tensor-parallel over heads across 8 NeuronCores. Core m owns q-heads
[4m, 4m+4) and kv-head m (GQA group-aligned), plus the matching o_proj column
slice; per-core partial outputs are summed on the host (the all-reduce).

Device layout notes:
 - qkv projection is computed transposed ([feature, t]) so attention works in
   the S^T = K^T.T @ Q^T layout; softmax sums over the partition axis are done
   with ones-vector matmuls on the PE, and PV consumes exp(S^T) directly.
 - RMSNorm scale and RoPE are fused via host-precomputed [128, T] cos/sin
   tables (norm weights + 1/sqrt(D) folded in); the partition-half rotation
   for RoPE uses SBUF->SBUF DMA.
 - All large matmuls use bf16 operands with fp32 PSUM accumulation.
 - Schedule per iteration: attention(tb) with o_proj(tb-1) matmuls injected
   every few units (o_proj has no exp dependency, so it fills the PE while
   the scalar engine streams exps), then the rms+rope chain for tb+1 (vector/
   gpsimd), then qkv matmuls for tb+2 whose dense PE phase hides that chain.
"""

import sys

import numpy as np

if "/opt/trn_rl_repo" not in sys.path:
    sys.path.insert(0, "/opt/trn_rl_repo")

import ml_dtypes

BF16 = ml_dtypes.bfloat16

HID = 4096
H = 32
HK = 8
D = 128
WIN = 1023
THETA = 1000000.0
EPS = 1e-6
SCALE = D ** -0.5
M = 8            # cores
QH = H // M      # q heads per core (4)
NJ = QH + 2      # j-blocks in qkv^T output (4 q + 1 k + 1 v)
TB = 512         # t free-dim block
NEG = -1.0e30
LAG = 3          # attention PV/rowsum lag behind QK (covers mask+exp latency)

_PROG_CACHE = {}


def _build_program(T):
    """Build the (single-core SPMD) Bass program for sequence length T."""
    from contextlib import ExitStack

    import concourse.bass as bass  # noqa: F401
    import concourse.tile as tile
    from concourse import bacc, mybir
    from concourse.masks import make_identity

    f32 = mybir.dt.float32
    bf = mybir.dt.bfloat16

    NT = T // TB          # number of t blocks
    NC = HID // 128       # contraction chunks
    NOB = HID // 128      # output row blocks

    nc = bacc.Bacc(
        "TRN2",
        target_bir_lowering=False,
        debug=False,
        enable_asserts=False,
        num_devices=M,
    )

    # x pre-tiled on host: block (tb, cq) = [128, 4*TB], 4 c-chunks interleaved
    # per partition row (4KB contiguous per partition per DMA)
    xT_h = nc.dram_tensor(
        "xT", [(T // TB) * (HID // 512) * 128, 4 * TB], bf, kind="ExternalInput"
    )
    wq_h = nc.dram_tensor("wqkvT", [HID, NJ * 128], bf, kind="ExternalInput")
    wo_h = nc.dram_tensor("woT", [QH * 128, HID], bf, kind="ExternalInput")
    cwq_h = nc.dram_tensor("cwq", [128, T], bf, kind="ExternalInput")
    swq_h = nc.dram_tensor("swq", [128, T], bf, kind="ExternalInput")
    cwk_h = nc.dram_tensor("cwk", [128, T], bf, kind="ExternalInput")
    swk_h = nc.dram_tensor("swk", [128, T], bf, kind="ExternalInput")
    maskd_h = nc.dram_tensor("maskd", [128, 128], f32, kind="ExternalInput")
    maskw_h = nc.dram_tensor("maskw", [128, 128], f32, kind="ExternalInput")
    # out pre-tiled: block (tb, obp) = [128, 2*TB] (ob pairs interleaved per row)
    outT_h = nc.dram_tensor(
        "outT", [(T // TB) * (HID // 256) * 128, 2 * TB], bf, kind="ExternalOutput"
    )

    xTr = xT_h.ap().rearrange("(b p) u -> b p u", p=128)
    wqr = wq_h.ap().rearrange("(c p) j -> p c j", p=128)
    wor = wo_h.ap().rearrange("(jc p) o -> p jc o", p=128)
    outr = outT_h.ap().rearrange("(b p) u -> b p u", p=128)

    mult = mybir.AluOpType.mult
    add = mybir.AluOpType.add
    Exp = mybir.ActivationFunctionType.Exp
    Sqrt = mybir.ActivationFunctionType.Sqrt

    with tile.TileContext(nc) as tc, ExitStack() as ctx:
        singles = ctx.enter_context(tc.tile_pool(name="singles", bufs=1))
        persist = ctx.enter_context(tc.tile_pool(name="persist", bufs=1))
        xpool = ctx.enter_context(tc.tile_pool(name="xpool", bufs=4))
        stpool = ctx.enter_context(tc.tile_pool(name="stpool", bufs=1))
        ropep = ctx.enter_context(tc.tile_pool(name="ropep", bufs=2))
        espool = ctx.enter_context(tc.tile_pool(name="espool", bufs=5))
        outp = ctx.enter_context(tc.tile_pool(name="outp", bufs=2))
        smallp = ctx.enter_context(tc.tile_pool(name="smallp", bufs=2))
        # PSUM: every tile is <= one bank; a single tag with 8 rotating slots
        # covers all 8 banks and lets phases overlap freely.
        psum = ctx.enter_context(tc.tile_pool(name="psum", bufs=8, space="PSUM"))
        drp = ctx.enter_context(tc.tile_pool(name="drp", bufs=4, space="DRAM"))

        def bcast_row(src_row, tag):
            """Broadcast a [1, TB] sbuf row to a [128, TB] sbuf tile.

            SBUF sources cannot have zero partition step in a DMA, so bounce
            through a DRAM scratch row and broadcast-read it back."""
            drs = drp.tile([1, TB], f32, name=f"drs_{tag}", tag=f"dr_{tag}", bufs=8)
            nc.gpsimd.dma_start(drs, src_row)
            dst = ropep.tile([128, TB], f32, name=f"bc_{tag}", tag=tag, bufs=6)
            nc.gpsimd.dma_start(dst, drs.to_broadcast([128, TB]))
            return dst

        # ---- cheap resident constants (no DMA) ----------------------------
        ident = singles.tile([128, 128], bf)
        make_identity(nc, ident)
        ones_bf = singles.tile([128, 1], bf)
        nc.vector.memset(ones_bf, 1.0)
        ones_row = singles.tile([1, 128], bf)
        nc.vector.memset(ones_row, 1.0)
        eps_sb = singles.tile([128, 1], f32)
        nc.vector.memset(eps_sb, EPS)

        maskd_sb = singles.tile([128, 128], f32)
        maskw_sb = singles.tile([128, 128], f32)
        w_sb = singles.tile([128, NC, NJ * 128], bf)
        cwq_sb = singles.tile([128, T], bf)
        swq_sb = singles.tile([128, T], bf)
        cwk_sb = singles.tile([128, T], bf)
        swk_sb = singles.tile([128, T], bf)
        wo_sb = singles.tile([128, QH, HID], bf)

        # ---- persistent activations ---------------------------------------
        qT = persist.tile([128, QH, T], bf)     # rope'd+normed q^T
        kT = persist.tile([128, T], bf)         # rope'd+normed k^T
        Vt = persist.tile([128, T // 128, 128], bf)  # v in [s, d] layout

        stages = {}

        def phase_a_mm(tb, startup=False):
            """qkv projection matmuls for t block tb (single x pass, 6 banks).

            On startup, interleave the w chunk DMAs with the x chunk DMAs on
            the sync queue so the PE starts ~4us in instead of waiting for
            the full 6.3MB weight load."""
            stage = stpool.tile(
                [128, NJ, TB], bf, tag="stage", bufs=2, name=f"stage_{tb}"
            )
            ps_all = [
                psum.tile([128, TB], f32, name=f"psqkv_{tb}_{j}", tag="bank")
                for j in range(NJ)
            ]
            if startup:
                # first 4 c-chunks of w land before the first x chunk
                nc.sync.dma_start(w_sb[:, 0:2, :], wqr[:, 0:2, :])
                nc.sync.dma_start(w_sb[:, 2:4, :], wqr[:, 2:4, :])
            for cq in range(NC // 4):
                xc = xpool.tile([128, 4, TB], bf, tag="xc", name=f"xc_{tb}_{cq}")
                nc.sync.dma_start(
                    xc,
                    xTr[tb * (NC // 4) + cq].rearrange("p (ci u) -> p ci u", u=TB),
                )
                if startup and cq + 1 < NC // 4:
                    c0 = 4 * (cq + 1)
                    nc.sync.dma_start(
                        w_sb[:, c0 : c0 + 2, :], wqr[:, c0 : c0 + 2, :]
                    )
                    nc.sync.dma_start(
                        w_sb[:, c0 + 2 : c0 + 4, :], wqr[:, c0 + 2 : c0 + 4, :]
                    )
                for ci in range(4):
                    c = 4 * cq + ci
                    for j in range(NJ):
                        nc.tensor.matmul(
                            ps_all[j],
                            lhsT=w_sb[:, c, j * 128 : (j + 1) * 128],
                            rhs=xc[:, ci, :],
                            start=(c == 0),
                            stop=(c == NC - 1),
                        )
            # PSUM -> SBUF casts on vector (keeps the scalar engine's
            # activation table parked on Exp/Sqrt)
            for j in range(NJ):
                nc.vector.tensor_copy(stage[:, j], ps_all[j])
            stages[tb] = stage

        sqs = {}
        scls = {}

        def a_tail_sq(tb):
            """x^2 tiles for tb's RMSNorm, emitted at iteration start so the
            vector engine produces them long before the ss matmuls run."""
            stage = stages[tb]
            tiles = []
            for j in range(QH + 1):
                sq = stpool.tile([128, TB], bf, tag="sq", bufs=5, name=f"sq_{tb}_{j}")
                nc.vector.tensor_tensor(sq, stage[:, j], stage[:, j], mult)
                tiles.append(sq)
            sqs[tb] = tiles

        def a_tail_mid(tb):
            """v transpose + rms reduce/scale + DRAM-bounce broadcast for tb.

            Emitted right after the attention/o_proj phase: the PE bits (v
            transposes, ss matmuls) depend only on tiles the vector engine
            finished during that phase; the bounce's DMA latency is hidden
            because the rope TTs consuming it run an iteration later."""
            stage = stages[tb]

            for u in range(TB // 128):
                ps_t = psum.tile([128, 128], bf, name=f"pst_{tb}_{u}", tag="bank")
                nc.tensor.transpose(ps_t, stage[:, QH + 1, u * 128 : (u + 1) * 128], ident)
                nc.vector.tensor_copy(Vt[:, tb * (TB // 128) + u, :], ps_t)

            ps_list = []
            for j in range(QH + 1):
                ps_ss = psum.tile([1, TB], f32, name=f"psss_{tb}_{j}", tag="bank")
                nc.tensor.matmul(ps_ss, lhsT=ones_bf, rhs=sqs[tb][j], start=True, stop=True)
                ps_list.append(ps_ss)
            bcs = []
            for j in range(QH + 1):
                rms = smallp.tile([1, TB], f32, tag="rms", name=f"rms_{tb}_{j}")
                nc.scalar.activation(rms, ps_list[j], Sqrt, bias=eps_sb[0:1, :], scale=1.0 / D)
                scl = smallp.tile([1, TB], f32, tag="scl", name=f"scl_{tb}_{j}")
                nc.vector.reciprocal_approx_fast(scl, rms)
                bcs.append(bcast_row(scl, "sclb"))
            del sqs[tb]
            scls[tb] = bcs

        def a_tail_rope(tb):
            """RoPE chain for tb (vector/gpsimd only; consumed by bc(tb) two
            iterations after its sq slice was emitted)."""
            t0 = tb * TB
            ts_ = slice(t0, t0 + TB)
            stage = stages.pop(tb)
            bcs = scls.pop(tb)
            for j in range(QH + 1):
                qn = ropep.tile([128, TB], bf, tag="qn", name=f"qn_{tb}_{j}")
                nc.vector.tensor_tensor(qn, stage[:, j], bcs[j], mult)
                qrot = ropep.tile([128, TB], bf, tag="qrot", name=f"qrot_{tb}_{j}")
                nc.gpsimd.dma_start(qrot[0:64, :], qn[64:128, :])
                nc.gpsimd.dma_start(qrot[64:128, :], qn[0:64, :])
                cw = cwq_sb if j < QH else cwk_sb
                sw = swq_sb if j < QH else swk_sb
                b_t = ropep.tile([128, TB], bf, tag="b_t", name=f"bt_{tb}_{j}")
                nc.gpsimd.tensor_tensor(b_t, qrot, sw[:, ts_], mult)
                nc.vector.tensor_tensor(qn, qn, cw[:, ts_], mult)
                dest = qT[:, j, ts_] if j < QH else kT[:, ts_]
                nc.gpsimd.tensor_tensor(dest, qn, b_t, add)

        attnTs = {}

        def phase_bc(tb, ctb):
            """attention for t block tb with o_proj for block ctb interleaved.

            Per head: QK/mask/exp runs LAG o-blocks ahead of PV/rowsum so the
            PE stream never waits on the vector/scalar exp chain. o_proj
            ob-pairs (pure PE work, no exp dependency) are injected every few
            units to keep the PE ahead of the scalar engine's exp rate."""
            t0 = tb * TB
            # o = sb - 4*tb; o=0 (full col range) goes FIRST so the
            # start=True PV/rowsum matmuls cover the whole bank; later
            # partial-range matmuls accumulate onto uniformly-written bytes
            # (CoreSim requires this; matches HW has_written semantics).
            obs = [0] + [o for o in range(-8, 4) if o != 0 and 4 * tb + o >= 0]
            attnT = outp.tile([128, QH, TB], bf, tag="attnT", name=f"attnT_{tb}")
            attnTs[tb] = attnT
            nob = len(obs)

            c_units = list(range(NOB // 2)) if ctb is not None else []
            cattnT = attnTs.pop(ctb) if ctb is not None else None
            n_units = QH * nob
            stride = max(1, (n_units + len(c_units) - 1) // max(1, len(c_units))) \
                if c_units else n_units + 1
            unit = 0

            def emit_c_unit(obp):
                o_st = outp.tile(
                    [128, 2, TB], bf, tag="o_st", bufs=3, name=f"ost_{ctb}_{obp}"
                )
                for oi2 in range(2):
                    ob = 2 * obp + oi2
                    ps_o = psum.tile(
                        [128, TB], f32, name=f"pso_{ctb}_{ob}", tag="bank"
                    )
                    for jc in range(QH):
                        nc.tensor.matmul(
                            ps_o,
                            lhsT=wo_sb[:, jc, ob * 128 : (ob + 1) * 128],
                            rhs=cattnT[:, jc, :],
                            start=(jc == 0),
                            stop=(jc == QH - 1),
                        )
                    nc.vector.tensor_copy(o_st[:, oi2, :], ps_o)
                nc.sync.dma_start(
                    outr[ctb * (NOB // 2) + obp].rearrange(
                        "p (oi u) -> p oi u", u=TB
                    ),
                    o_st,
                )

            def tick():
                nonlocal unit
                unit += 1
                if c_units and unit % stride == 0:
                    emit_c_unit(c_units.pop(0))

            for h in range(QH):
                pv = psum.tile([128, TB], f32, name=f"pspv_{tb}_{h}", tag="bank")
                rs = psum.tile([1, TB], f32, name=f"psr_{tb}_{h}", tag="bank")
                ess = {}
                rngs = {}

                def emit_pv(oi):
                    o = obs[oi]
                    c0, c1 = rngs[oi]
                    first = oi == 0
                    last = oi == nob - 1
                    nc.tensor.matmul(
                        pv[:, c0:c1],
                        lhsT=Vt[:, 4 * tb + o, :],
                        rhs=ess[oi][:, c0:c1],
                        start=first,
                        stop=last,
                        skip_group_check=True,
                    )
                    nc.tensor.matmul(
                        rs[:, c0:c1],
                        lhsT=ones_bf,
                        rhs=ess[oi][:, c0:c1],
                        start=first,
                        stop=last,
                        skip_group_check=True,
                    )
                    del ess[oi]

                for oi, o in enumerate(obs):
                    sb = 4 * tb + o
                    if o >= 0:
                        c0, c1 = 128 * o, TB
                    elif o >= -4:
                        c0, c1 = 0, TB
                    else:
                        c0, c1 = 0, 128 * (o + 9)
                    rngs[oi] = (c0, c1)
                    ps_s = psum.tile(
                        [128, TB], f32, name=f"pss_{tb}_{h}_{oi}", tag="bank"
                    )
                    nc.tensor.matmul(
                        ps_s[:, c0:c1],
                        lhsT=kT[:, sb * 128 : (sb + 1) * 128],
                        rhs=qT[:, h, t0 + c0 : t0 + c1],
                        start=True,
                        stop=True,
                    )
                    if o >= 0:  # causal strip at cols [128o, 128o+128)
                        u0 = 128 * o
                        nc.vector.tensor_tensor(
                            ps_s[:, u0 : u0 + 128], ps_s[:, u0 : u0 + 128],
                            maskd_sb, add,
                        )
                    elif o <= -5:  # window strip
                        u0 = 128 * (o + 8)
                        nc.vector.tensor_tensor(
                            ps_s[:, u0 : u0 + 128], ps_s[:, u0 : u0 + 128],
                            maskw_sb, add,
                        )
                    es = espool.tile(
                        [128, TB], bf, tag="es", name=f"es_{tb}_{h}_{oi}"
                    )
                    nc.scalar.activation(es[:, c0:c1], ps_s[:, c0:c1], Exp)
                    ess[oi] = es
                    if oi >= LAG:
                        emit_pv(oi - LAG)
                    tick()
                for oi in range(max(0, nob - LAG), nob):
                    emit_pv(oi)

                # normalize: attnT[:, h] = pv * (1/rowsum) broadcast
                rsum = smallp.tile([1, TB], f32, tag="rsum", name=f"rsum_{tb}_{h}")
                nc.vector.tensor_copy(rsum, rs)
                nc.vector.reciprocal_approx_fast(rsum, rsum)
                rb = bcast_row(rsum, "sclb")
                nc.vector.tensor_tensor(attnT[:, h, :], pv, rb, mult)

            # leftover o_proj units (ragged stride)
            while c_units:
                emit_c_unit(c_units.pop(0))

        def phase_c(tb):
            """o_proj for t block tb standalone (epilogue)."""
            attnT = attnTs.pop(tb)
            for obp in range(NOB // 2):
                o_st = outp.tile(
                    [128, 2, TB], bf, tag="o_st", bufs=3, name=f"ost_{tb}_{obp}"
                )
                for oi in range(2):
                    ob = 2 * obp + oi
                    ps_o = psum.tile([128, TB], f32, name=f"pso_{tb}_{ob}", tag="bank")
                    for jc in range(QH):
                        nc.tensor.matmul(
                            ps_o,
                            lhsT=wo_sb[:, jc, ob * 128 : (ob + 1) * 128],
                            rhs=attnT[:, jc, :],
                            start=(jc == 0),
                            stop=(jc == QH - 1),
                        )
                    if oi == 0:
                        nc.vector.tensor_copy(o_st[:, oi, :], ps_o)
                    else:
                        nc.scalar.copy(o_st[:, oi, :], ps_o)
                nc.sync.dma_start(
                    outr[tb * (NOB // 2) + obp].rearrange("p (oi u) -> p oi u", u=TB),
                    o_st,
                )

        # Software pipeline (see module docstring). Per iteration, each
        # engine's queue is ordered so every consumer sits after work that
        # its producers overlap: sq TTs first, then bc's masks/copies, then
        # the rms chain, then A_mm's matmuls, then the rope TTs.
        phase_a_mm(0, startup=True)
        nc.sync.dma_start(cwq_sb, cwq_h.ap())
        nc.sync.dma_start(swq_sb, swq_h.ap())
        nc.sync.dma_start(cwk_sb, cwk_h.ap())
        nc.sync.dma_start(swk_sb, swk_h.ap())
        a_tail_sq(0)
        a_tail_mid(0)
        phase_a_mm(1)
        nc.sync.dma_start(maskd_sb, maskd_h.ap())
        nc.sync.dma_start(maskw_sb, maskw_h.ap())
        nc.sync.dma_start(wo_sb, wor)
        a_tail_rope(0)
        a_tail_sq(1)
        a_tail_mid(1)
        phase_a_mm(2)
        a_tail_rope(1)
        for tb in range(NT):
            if tb + 2 < NT:
                a_tail_sq(tb + 2)
            phase_bc(tb, tb - 1 if tb >= 1 else None)
            if tb + 2 < NT:
                a_tail_mid(tb + 2)
            if tb + 3 < NT:
                phase_a_mm(tb + 3)
            if tb + 2 < NT:
                a_tail_rope(tb + 2)
        phase_c(NT - 1)

    nc.compile()
    return nc


def _get_program(T):
    if T not in _PROG_CACHE:
        _PROG_CACHE[T] = _build_program(T)
    return _PROG_CACHE[T]


def _host_prep(positions, hidden_states, wqkv, wo, q_norm_w, k_norm_w):
    """Build the 8 per-core input maps (host-side sharding + table prep)."""
    T = hidden_states.shape[0]
    pos = np.asarray(positions).astype(np.float64)
    hs = np.asarray(hidden_states, dtype=np.float32)
    wqkv = np.asarray(wqkv, dtype=np.float32)
    wo = np.asarray(wo, dtype=np.float32)
    qw = np.asarray(q_norm_w, dtype=np.float64)
    kw = np.asarray(k_norm_w, dtype=np.float64)

    half = D // 2
    inv_freq = 1.0 / (THETA ** (np.arange(0, D, 2, dtype=np.float64) / D))  # [64]
    th = pos[:, None] * inv_freq[None, :]          # [T, 64]
    cos = np.cos(th).T                             # [64, T] float64
    sin = np.sin(th).T

    def tables(w, scale):
        cw = np.empty((D, T), np.float64)
        sw = np.empty((D, T), np.float64)
        cw[:half] = cos * (w[:half, None] * scale)
        cw[half:] = cos * (w[half:, None] * scale)
        # out[d<64] = qn[d]*w[d]*cos - qn[d+64]*w[d+64]*sin  (rot reads qn[d+64])
        sw[:half] = -sin * (w[half:, None] * scale)
        # out[d>=64] = qn[d]*w[d]*cos + qn[d-64]*w[d-64]*sin
        sw[half:] = sin * (w[:half, None] * scale)
        return cw.astype(BF16), sw.astype(BF16)

    cwq, swq = tables(qw, SCALE)
    cwk, swk = tables(kw, 1.0)

    si = np.arange(128)[:, None]
    ui = np.arange(128)[None, :]
    maskd = np.where(ui >= si, 0.0, NEG).astype(np.float32)
    maskw = np.where(ui < si, 0.0, NEG).astype(np.float32)

    # tiled layout: block (tb, cq) = [128, 4*TB]; row p holds c-chunks
    # 4cq..4cq+3 back to back (4KB contiguous per partition)
    NTb, NCq = T // TB, HID // 512
    xT = np.ascontiguousarray(
        hs.T.reshape(NCq, 4, 128, NTb, TB)
        .transpose(3, 0, 2, 1, 4)
        .reshape(NTb * NCq * 128, 4 * TB)
    ).astype(BF16)

    in_maps = []
    for m in range(M):
        wq_m = wqkv[m * QH * D : (m + 1) * QH * D]            # [512, HID]
        wk_m = wqkv[H * D + m * D : H * D + (m + 1) * D]      # [128, HID]
        wv_m = wqkv[(H + HK) * D + m * D : (H + HK) * D + (m + 1) * D]
        wqkvT_m = np.ascontiguousarray(
            np.concatenate([wq_m, wk_m, wv_m], axis=0).T
        ).astype(BF16)                                        # [HID, 768]
        woT_m = np.ascontiguousarray(
            wo[:, m * QH * D : (m + 1) * QH * D].T
        ).astype(BF16)                                        # [512, HID]
        in_maps.append(
            {
                "xT": xT,
                "wqkvT": wqkvT_m,
                "woT": woT_m,
                "cwq": cwq,
                "swq": swq,
                "cwk": cwk,
                "swk": swk,
                "maskd": maskd,
                "maskw": maskw,
            }
        )
    return in_maps


def _run(in_maps, T, trace=False):
    from concourse import bass_utils

    nc = _get_program(T)
    res = bass_utils.run_bass_kernel_spmd(
        nc, in_maps, core_ids=list(range(M)), trace=trace
    )
    return res


def kernel(positions, hidden_states, wqkv, wo, q_norm_w, k_norm_w, _trace=False):
    T = hidden_states.shape[0]
    in_maps = _host_prep(positions, hidden_states, wqkv, wo, q_norm_w, k_norm_w)
    res = _run(in_maps, T, trace=_trace)
    NTb, NOBp = T // TB, HID // 256
    acc = np.zeros((NTb, NOBp, 128, 2, TB), np.float64)
    for r in res.results:
        acc += r["outT"].astype(np.float64).reshape(NTb, NOBp, 128, 2, TB)
    # untile: out[t, o] with o = (2*obp + oi)*128 + p, t = tb*TB + u
    out = np.ascontiguousarray(
        acc.transpose(0, 4, 1, 3, 2).reshape(T, HID)
    ).astype(np.float32)
    kernel._last_results = res
    return out


# revision 17
# speedup vs baseline: 1.2091x; 1.0021x over previous
"""Trainium2 Bass kernel for Exaone4-style GQA attention block (T=2048, HID=4096,
H=32 q-heads, HK=8 kv-heads, D=128, sliding window 1023, QK-RMSNorm + NeoX RoPE).

Sharding: tensor-parallel over heads across 8 NeuronCores. Core m owns q-heads
[4m, 4m+4) and kv-head m (GQA group-aligned), plus the matching o_proj column
slice; per-core partial outputs are summed on the host (the all-reduce).

Device layout notes:
 - qkv projection is computed transposed ([feature, t]) so attention works in
   the S^T = K^T.T @ Q^T layout; softmax sums over the partition axis are done
   with ones-vector matmuls on the PE, and PV consumes exp(S^T) directly.
 - RMSNorm scale and RoPE are fused via host-precomputed [128, T] cos/sin
   tables (norm weights + 1/sqrt(D) folded in); the partition-half rotation
   for RoPE uses SBUF->SBUF DMA.
 - All large matmuls use bf16 operands with fp32 PSUM accumulation.
 - Schedule per iteration: attention(tb) with o_proj(tb-1) matmuls injected
   every few units (o_proj has no exp dependency, so it fills the PE while
   the scalar engine streams exps), then the rms+rope chain for tb+1 (vector/
   gpsimd), then qkv matmuls for tb+2 whose dense PE phase hides that chain.
"""

import sys

import numpy as np

if "/opt/trn_rl_repo" not in sys.path:
    sys.path.insert(0, "/opt/trn_rl_repo")

import ml_dtypes

BF16 = ml_dtypes.bfloat16

HID = 4096
H = 32
HK = 8
D = 128
WIN = 1023
THETA = 1000000.0
EPS = 1e-6
SCALE = D ** -0.5
M = 8            # cores
QH = H // M      # q heads per core (4)
NJ = QH + 2      # j-blocks in qkv^T output (4 q + 1 k + 1 v)
TB = 512         # t free-dim block
NEG = -1.0e30
LAG = 3          # attention PV/rowsum lag behind QK (covers mask+exp latency)

_PROG_CACHE = {}


def _build_program(T):
    """Build the (single-core SPMD) Bass program for sequence length T."""
    from contextlib import ExitStack

    import concourse.bass as bass  # noqa: F401
    import concourse.tile as tile
    from concourse import bacc, mybir
    from concourse.masks import make_identity

    f32 = mybir.dt.float32
    bf = mybir.dt.bfloat16

    NT = T // TB          # number of t blocks
    NC = HID // 128       # contraction chunks
    NOB = HID // 128      # output row blocks

    nc = bacc.Bacc(
        "TRN2",
        target_bir_lowering=False,
        debug=False,
        enable_asserts=False,
        num_devices=M,
    )

    # x pre-tiled on host: block (tb, cq) = [128, 4*TB], 4 c-chunks interleaved
    # per partition row (4KB contiguous per partition per DMA)
    xT_h = nc.dram_tensor(
        "xT", [(T // TB) * (HID // 512) * 128, 4 * TB], bf, kind="ExternalInput"
    )
    wq_h = nc.dram_tensor("wqkvT", [HID, NJ * 128], bf, kind="ExternalInput")
    wo_h = nc.dram_tensor("woT", [QH * 128, HID], bf, kind="ExternalInput")
    cwq_h = nc.dram_tensor("cwq", [128, T], bf, kind="ExternalInput")
    swq_h = nc.dram_tensor("swq", [128, T], bf, kind="ExternalInput")
    cwk_h = nc.dram_tensor("cwk", [128, T], bf, kind="ExternalInput")
    swk_h = nc.dram_tensor("swk", [128, T], bf, kind="ExternalInput")
    maskd_h = nc.dram_tensor("maskd", [128, 128], f32, kind="ExternalInput")
    maskw_h = nc.dram_tensor("maskw", [128, 128], f32, kind="ExternalInput")
    # out pre-tiled: block (tb, obp) = [128, 2*TB] (ob pairs interleaved per row)
    outT_h = nc.dram_tensor(
        "outT", [(T // TB) * (HID // 256) * 128, 2 * TB], bf, kind="ExternalOutput"
    )

    xTr = xT_h.ap().rearrange("(b p) u -> b p u", p=128)
    wqr = wq_h.ap().rearrange("(c p) j -> p c j", p=128)
    wor = wo_h.ap().rearrange("(jc p) o -> p jc o", p=128)
    outr = outT_h.ap().rearrange("(b p) u -> b p u", p=128)

    mult = mybir.AluOpType.mult
    add = mybir.AluOpType.add
    Exp = mybir.ActivationFunctionType.Exp
    Sqrt = mybir.ActivationFunctionType.Sqrt

    with tile.TileContext(nc) as tc, ExitStack() as ctx:
        singles = ctx.enter_context(tc.tile_pool(name="singles", bufs=1))
        persist = ctx.enter_context(tc.tile_pool(name="persist", bufs=1))
        xpool = ctx.enter_context(tc.tile_pool(name="xpool", bufs=4))
        stpool = ctx.enter_context(tc.tile_pool(name="stpool", bufs=1))
        ropep = ctx.enter_context(tc.tile_pool(name="ropep", bufs=2))
        espool = ctx.enter_context(tc.tile_pool(name="espool", bufs=5))
        outp = ctx.enter_context(tc.tile_pool(name="outp", bufs=2))
        smallp = ctx.enter_context(tc.tile_pool(name="smallp", bufs=2))
        # PSUM: every tile is <= one bank; a single tag with 8 rotating slots
        # covers all 8 banks and lets phases overlap freely.
        psum = ctx.enter_context(tc.tile_pool(name="psum", bufs=8, space="PSUM"))
        drp = ctx.enter_context(tc.tile_pool(name="drp", bufs=4, space="DRAM"))

        def bcast_row(src_row, tag):
            """Broadcast a [1, TB] sbuf row to a [128, TB] sbuf tile.

            SBUF sources cannot have zero partition step in a DMA, so bounce
            through a DRAM scratch row and broadcast-read it back."""
            drs = drp.tile([1, TB], f32, name=f"drs_{tag}", tag=f"dr_{tag}", bufs=8)
            nc.gpsimd.dma_start(drs, src_row)
            dst = ropep.tile([128, TB], f32, name=f"bc_{tag}", tag=tag, bufs=6)
            nc.gpsimd.dma_start(dst, drs.to_broadcast([128, TB]))
            return dst

        # ---- cheap resident constants (no DMA) ----------------------------
        ident = singles.tile([128, 128], bf)
        make_identity(nc, ident)
        ones_bf = singles.tile([128, 1], bf)
        nc.vector.memset(ones_bf, 1.0)
        ones_row = singles.tile([1, 128], bf)
        nc.vector.memset(ones_row, 1.0)
        eps_sb = singles.tile([128, 1], f32)
        nc.vector.memset(eps_sb, EPS)

        maskd_sb = singles.tile([128, 128], f32)
        maskw_sb = singles.tile([128, 128], f32)
        w_sb = singles.tile([128, NC, NJ * 128], bf)
        cwq_sb = singles.tile([128, T], bf)
        swq_sb = singles.tile([128, T], bf)
        cwk_sb = singles.tile([128, T], bf)
        swk_sb = singles.tile([128, T], bf)
        wo_sb = singles.tile([128, QH, HID], bf)

        # ---- persistent activations ---------------------------------------
        qT = persist.tile([128, QH, T], bf)     # rope'd+normed q^T
        kT = persist.tile([128, T], bf)         # rope'd+normed k^T
        Vt = persist.tile([128, T // 128, 128], bf)  # v in [s, d] layout

        stages = {}

        def phase_a_mm(tb, startup=False):
            """qkv projection matmuls for t block tb (single x pass, 6 banks).

            On startup, interleave the w chunk DMAs with the x chunk DMAs on
            the sync queue so the PE starts ~4us in instead of waiting for
            the full 6.3MB weight load."""
            stage = stpool.tile(
                [128, NJ, TB], bf, tag="stage", bufs=2, name=f"stage_{tb}"
            )
            ps_all = [
                psum.tile([128, TB], f32, name=f"psqkv_{tb}_{j}", tag="bank")
                for j in range(NJ)
            ]
            if startup:
                # first 4 c-chunks of w land before the first x chunk
                nc.sync.dma_start(w_sb[:, 0:2, :], wqr[:, 0:2, :])
                nc.sync.dma_start(w_sb[:, 2:4, :], wqr[:, 2:4, :])
            for cq in range(NC // 4):
                xc = xpool.tile([128, 4, TB], bf, tag="xc", name=f"xc_{tb}_{cq}")
                nc.sync.dma_start(
                    xc,
                    xTr[tb * (NC // 4) + cq].rearrange("p (ci u) -> p ci u", u=TB),
                )
                if startup and cq + 1 < NC // 4:
                    c0 = 4 * (cq + 1)
                    nc.sync.dma_start(
                        w_sb[:, c0 : c0 + 2, :], wqr[:, c0 : c0 + 2, :]
                    )
                    nc.sync.dma_start(
                        w_sb[:, c0 + 2 : c0 + 4, :], wqr[:, c0 + 2 : c0 + 4, :]
                    )
                for ci in range(4):
                    c = 4 * cq + ci
                    for j in range(NJ):
                        nc.tensor.matmul(
                            ps_all[j],
                            lhsT=w_sb[:, c, j * 128 : (j + 1) * 128],
                            rhs=xc[:, ci, :],
                            start=(c == 0),
                            stop=(c == NC - 1),
                        )
            # PSUM -> SBUF casts on vector (keeps the scalar engine's
            # activation table parked on Exp/Sqrt); v block first so the
            # transposes that follow never wait on the copy queue
            for j in [NJ - 1] + list(range(NJ - 1)):
                nc.vector.tensor_copy(stage[:, j], ps_all[j])
            stages[tb] = stage

        sqs = {}
        scls = {}

        def a_tail_sq(tb):
            """x^2 tiles for tb's RMSNorm, emitted at iteration start so the
            vector engine produces them long before the ss matmuls run."""
            stage = stages[tb]
            tiles = []
            for j in range(QH + 1):
                sq = stpool.tile([128, TB], bf, tag="sq", bufs=5, name=f"sq_{tb}_{j}")
                nc.vector.tensor_tensor(sq, stage[:, j], stage[:, j], mult)
                tiles.append(sq)
            sqs[tb] = tiles

        def a_tail_mid(tb):
            """v transpose + rms reduce/scale + DRAM-bounce broadcast for tb.

            Emitted right after the attention/o_proj phase: the PE bits (v
            transposes, ss matmuls) depend only on tiles the vector engine
            finished during that phase; the bounce's DMA latency is hidden
            because the rope TTs consuming it run an iteration later."""
            stage = stages[tb]

            for u in range(TB // 128):
                ps_t = psum.tile([128, 128], bf, name=f"pst_{tb}_{u}", tag="bank")
                nc.tensor.transpose(ps_t, stage[:, QH + 1, u * 128 : (u + 1) * 128], ident)
                nc.vector.tensor_copy(Vt[:, tb * (TB // 128) + u, :], ps_t)

            ps_list = []
            for j in range(QH + 1):
                ps_ss = psum.tile([1, TB], f32, name=f"psss_{tb}_{j}", tag="bank")
                nc.tensor.matmul(ps_ss, lhsT=ones_bf, rhs=sqs[tb][j], start=True, stop=True)
                ps_list.append(ps_ss)
            bcs = []
            for j in range(QH + 1):
                rms = smallp.tile([1, TB], f32, tag="rms", name=f"rms_{tb}_{j}")
                nc.scalar.activation(rms, ps_list[j], Sqrt, bias=eps_sb[0:1, :], scale=1.0 / D)
                scl = smallp.tile([1, TB], f32, tag="scl", name=f"scl_{tb}_{j}")
                nc.vector.reciprocal_approx_fast(scl, rms)
                bcs.append(bcast_row(scl, "sclb"))
            del sqs[tb]
            scls[tb] = bcs

        def a_tail_rope(tb):
            """RoPE chain for tb (vector/gpsimd only; consumed by bc(tb) two
            iterations after its sq slice was emitted)."""
            t0 = tb * TB
            ts_ = slice(t0, t0 + TB)
            stage = stages.pop(tb)
            bcs = scls.pop(tb)
            for j in range(QH + 1):
                qn = ropep.tile([128, TB], bf, tag="qn", name=f"qn_{tb}_{j}")
                nc.vector.tensor_tensor(qn, stage[:, j], bcs[j], mult)
                qrot = ropep.tile([128, TB], bf, tag="qrot", name=f"qrot_{tb}_{j}")
                nc.gpsimd.dma_start(qrot[0:64, :], qn[64:128, :])
                nc.gpsimd.dma_start(qrot[64:128, :], qn[0:64, :])
                cw = cwq_sb if j < QH else cwk_sb
                sw = swq_sb if j < QH else swk_sb
                b_t = ropep.tile([128, TB], bf, tag="b_t", name=f"bt_{tb}_{j}")
                nc.gpsimd.tensor_tensor(b_t, qrot, sw[:, ts_], mult)
                nc.vector.tensor_tensor(qn, qn, cw[:, ts_], mult)
                dest = qT[:, j, ts_] if j < QH else kT[:, ts_]
                nc.gpsimd.tensor_tensor(dest, qn, b_t, add)

        attnTs = {}

        def phase_bc(tb, ctb, ntb=None):
            """attention for t block tb with o_proj for block ctb interleaved.

            Per head: QK/mask/exp runs LAG o-blocks ahead of PV/rowsum so the
            PE stream never waits on the vector/scalar exp chain. o_proj
            ob-pairs (pure PE work, no exp dependency) are injected every few
            units to keep the PE ahead of the scalar engine's exp rate."""
            t0 = tb * TB
            # o = sb - 4*tb; o=0 (full col range) goes FIRST so the
            # start=True PV/rowsum matmuls cover the whole bank; later
            # partial-range matmuls accumulate onto uniformly-written bytes
            # (CoreSim requires this; matches HW has_written semantics).
            obs = [0] + [o for o in range(-8, 4) if o != 0 and 4 * tb + o >= 0]
            attnT = outp.tile([128, QH, TB], bf, tag="attnT", name=f"attnT_{tb}")
            attnTs[tb] = attnT
            nob = len(obs)

            c_units = list(range(NOB // 2)) if ctb is not None else []
            cattnT = attnTs.pop(ctb) if ctb is not None else None
            n_units = QH * nob
            stride = max(1, (n_units + len(c_units) - 1) // max(1, len(c_units))) \
                if c_units else n_units + 1
            unit = 0

            def emit_c_unit(obp):
                o_st = outp.tile(
                    [128, 2, TB], bf, tag="o_st", bufs=3, name=f"ost_{ctb}_{obp}"
                )
                for oi2 in range(2):
                    ob = 2 * obp + oi2
                    ps_o = psum.tile(
                        [128, TB], f32, name=f"pso_{ctb}_{ob}", tag="bank"
                    )
                    for jc in range(QH):
                        nc.tensor.matmul(
                            ps_o,
                            lhsT=wo_sb[:, jc, ob * 128 : (ob + 1) * 128],
                            rhs=cattnT[:, jc, :],
                            start=(jc == 0),
                            stop=(jc == QH - 1),
                        )
                    nc.vector.tensor_copy(o_st[:, oi2, :], ps_o)
                nc.sync.dma_start(
                    outr[ctb * (NOB // 2) + obp].rearrange(
                        "p (oi u) -> p oi u", u=TB
                    ),
                    o_st,
                )

            sq_units = list(range(QH + 1)) if ntb is not None else []

            def tick():
                nonlocal unit
                unit += 1
                if sq_units and unit % 2 == 0 and unit <= 2 * (QH + 1):
                    # one x^2 TT for tb+2's rmsnorm, slotted between masks so
                    # the first masks of this block aren't delayed
                    j = sq_units.pop(0)
                    stg = stages[ntb]
                    sq = stpool.tile(
                        [128, TB], bf, tag="sq", bufs=5, name=f"sq_{ntb}_{j}"
                    )
                    nc.vector.tensor_tensor(sq, stg[:, j], stg[:, j], mult)
                    sqs.setdefault(ntb, []).append(sq)
                if c_units and unit % stride == 0:
                    emit_c_unit(c_units.pop(0))

            for h in range(QH):
                pv = psum.tile([128, TB], f32, name=f"pspv_{tb}_{h}", tag="bank")
                rs = psum.tile([1, TB], f32, name=f"psr_{tb}_{h}", tag="bank")
                ess = {}
                rngs = {}

                def emit_pv(oi):
                    o = obs[oi]
                    c0, c1 = rngs[oi]
                    first = oi == 0
                    last = oi == nob - 1
                    nc.tensor.matmul(
                        pv[:, c0:c1],
                        lhsT=Vt[:, 4 * tb + o, :],
                        rhs=ess[oi][:, c0:c1],
                        start=first,
                        stop=last,
                        skip_group_check=True,
                    )
                    nc.tensor.matmul(
                        rs[:, c0:c1],
                        lhsT=ones_bf,
                        rhs=ess[oi][:, c0:c1],
                        start=first,
                        stop=last,
                        skip_group_check=True,
                    )
                    del ess[oi]

                for oi, o in enumerate(obs):
                    sb = 4 * tb + o
                    if o >= 0:
                        c0, c1 = 128 * o, TB
                    elif o >= -4:
                        c0, c1 = 0, TB
                    else:
                        c0, c1 = 0, 128 * (o + 9)
                    rngs[oi] = (c0, c1)
                    ps_s = psum.tile(
                        [128, TB], f32, name=f"pss_{tb}_{h}_{oi}", tag="bank"
                    )
                    nc.tensor.matmul(
                        ps_s[:, c0:c1],
                        lhsT=kT[:, sb * 128 : (sb + 1) * 128],
                        rhs=qT[:, h, t0 + c0 : t0 + c1],
                        start=True,
                        stop=True,
                    )
                    if o >= 0:  # causal strip at cols [128o, 128o+128)
                        u0 = 128 * o
                        nc.vector.tensor_tensor(
                            ps_s[:, u0 : u0 + 128], ps_s[:, u0 : u0 + 128],
                            maskd_sb, add,
                        )
                    elif o <= -5:  # window strip
                        u0 = 128 * (o + 8)
                        nc.vector.tensor_tensor(
                            ps_s[:, u0 : u0 + 128], ps_s[:, u0 : u0 + 128],
                            maskw_sb, add,
                        )
                    es = espool.tile(
                        [128, TB], bf, tag="es", name=f"es_{tb}_{h}_{oi}"
                    )
                    nc.scalar.activation(es[:, c0:c1], ps_s[:, c0:c1], Exp)
                    ess[oi] = es
                    if oi >= LAG:
                        emit_pv(oi - LAG)
                    tick()
                for oi in range(max(0, nob - LAG), nob):
                    emit_pv(oi)

                # normalize: attnT[:, h] = pv * (1/rowsum) broadcast
                rsum = smallp.tile([1, TB], f32, tag="rsum", name=f"rsum_{tb}_{h}")
                nc.vector.tensor_copy(rsum, rs)
                nc.vector.reciprocal_approx_fast(rsum, rsum)
                rb = bcast_row(rsum, "sclb")
                nc.vector.tensor_tensor(attnT[:, h, :], pv, rb, mult)

            # leftover o_proj units (ragged stride)
            while c_units:
                emit_c_unit(c_units.pop(0))

        def phase_c(tb):
            """o_proj for t block tb standalone (epilogue)."""
            attnT = attnTs.pop(tb)
            for obp in range(NOB // 2):
                o_st = outp.tile(
                    [128, 2, TB], bf, tag="o_st", bufs=3, name=f"ost_{tb}_{obp}"
                )
                for oi in range(2):
                    ob = 2 * obp + oi
                    ps_o = psum.tile([128, TB], f32, name=f"pso_{tb}_{ob}", tag="bank")
                    for jc in range(QH):
                        nc.tensor.matmul(
                            ps_o,
                            lhsT=wo_sb[:, jc, ob * 128 : (ob + 1) * 128],
                            rhs=attnT[:, jc, :],
                            start=(jc == 0),
                            stop=(jc == QH - 1),
                        )
                    if oi == 0:
                        nc.vector.tensor_copy(o_st[:, oi, :], ps_o)
                    else:
                        nc.scalar.copy(o_st[:, oi, :], ps_o)
                nc.sync.dma_start(
                    outr[tb * (NOB // 2) + obp].rearrange("p (oi u) -> p oi u", u=TB),
                    o_st,
                )

        # Software pipeline (see module docstring). Per iteration, each
        # engine's queue is ordered so every consumer sits after work that
        # its producers overlap: sq TTs first, then bc's masks/copies, then
        # the rms chain, then A_mm's matmuls, then the rope TTs.
        phase_a_mm(0, startup=True)
        nc.sync.dma_start(cwq_sb, cwq_h.ap())
        nc.sync.dma_start(swq_sb, swq_h.ap())
        nc.sync.dma_start(cwk_sb, cwk_h.ap())
        nc.sync.dma_start(swk_sb, swk_h.ap())
        a_tail_sq(0)
        a_tail_mid(0)
        phase_a_mm(1)
        nc.sync.dma_start(maskd_sb, maskd_h.ap())
        nc.sync.dma_start(maskw_sb, maskw_h.ap())
        nc.sync.dma_start(wo_sb, wor)
        a_tail_rope(0)
        a_tail_sq(1)
        a_tail_mid(1)
        phase_a_mm(2)
        a_tail_rope(1)
        for tb in range(NT):
            phase_bc(tb, tb - 1 if tb >= 1 else None,
                     tb + 2 if tb + 2 < NT else None)
            if tb + 2 < NT:
                a_tail_mid(tb + 2)
            if tb + 3 < NT:
                phase_a_mm(tb + 3)
            if tb + 2 < NT:
                a_tail_rope(tb + 2)
        phase_c(NT - 1)

    nc.compile()
    return nc


def _get_program(T):
    if T not in _PROG_CACHE:
        _PROG_CACHE[T] = _build_program(T)
    return _PROG_CACHE[T]


def _host_prep(positions, hidden_states, wqkv, wo, q_norm_w, k_norm_w):
    """Build the 8 per-core input maps (host-side sharding + table prep)."""
    T = hidden_states.shape[0]
    pos = np.asarray(positions).astype(np.float64)
    hs = np.asarray(hidden_states, dtype=np.float32)
    wqkv = np.asarray(wqkv, dtype=np.float32)
    wo = np.asarray(wo, dtype=np.float32)
    qw = np.asarray(q_norm_w, dtype=np.float64)
    kw = np.asarray(k_norm_w, dtype=np.float64)

    half = D // 2
    inv_freq = 1.0 / (THETA ** (np.arange(0, D, 2, dtype=np.float64) / D))  # [64]
    th = pos[:, None] * inv_freq[None, :]          # [T, 64]
    cos = np.cos(th).T                             # [64, T] float64
    sin = np.sin(th).T

    def tables(w, scale):
        cw = np.empty((D, T), np.float64)
        sw = np.empty((D, T), np.float64)
        cw[:half] = cos * (w[:half, None] * scale)
        cw[half:] = cos * (w[half:, None] * scale)
        # out[d<64] = qn[d]*w[d]*cos - qn[d+64]*w[d+64]*sin  (rot reads qn[d+64])
        sw[:half] = -sin * (w[half:, None] * scale)
        # out[d>=64] = qn[d]*w[d]*cos + qn[d-64]*w[d-64]*sin
        sw[half:] = sin * (w[:half, None] * scale)
        return cw.astype(BF16), sw.astype(BF16)

    cwq, swq = tables(qw, SCALE)
    cwk, swk = tables(kw, 1.0)

    si = np.arange(128)[:, None]
    ui = np.arange(128)[None, :]
    maskd = np.where(ui >= si, 0.0, NEG).astype(np.float32)
    maskw = np.where(ui < si, 0.0, NEG).astype(np.float32)

    # tiled layout: block (tb, cq) = [128, 4*TB]; row p holds c-chunks
    # 4cq..4cq+3 back to back (4KB contiguous per partition)
    NTb, NCq = T // TB, HID // 512
    xT = np.ascontiguousarray(
        hs.T.reshape(NCq, 4, 128, NTb, TB)
        .transpose(3, 0, 2, 1, 4)
        .reshape(NTb * NCq * 128, 4 * TB)
    ).astype(BF16)

    in_maps = []
    for m in range(M):
        wq_m = wqkv[m * QH * D : (m + 1) * QH * D]            # [512, HID]
        wk_m = wqkv[H * D + m * D : H * D + (m + 1) * D]      # [128, HID]
        wv_m = wqkv[(H + HK) * D + m * D : (H + HK) * D + (m + 1) * D]
        wqkvT_m = np.ascontiguousarray(
            np.concatenate([wq_m, wk_m, wv_m], axis=0).T
        ).astype(BF16)                                        # [HID, 768]
        woT_m = np.ascontiguousarray(
            wo[:, m * QH * D : (m + 1) * QH * D].T
        ).astype(BF16)                                        # [512, HID]
        in_maps.append(
            {
                "xT": xT,
                "wqkvT": wqkvT_m,
                "woT": woT_m,
                "cwq": cwq,
                "swq": swq,
                "cwk": cwk,
                "swk": swk,
                "maskd": maskd,
                "maskw": maskw,
            }
        )
    return in_maps


def _run(in_maps, T, trace=False):
    from concourse import bass_utils

    nc = _get_program(T)
    res = bass_utils.run_bass_kernel_spmd(
        nc, in_maps, core_ids=list(range(M)), trace=trace
    )
    return res


def kernel(positions, hidden_states, wqkv, wo, q_norm_w, k_norm_w, _trace=False):
    T = hidden_states.shape[0]
    in_maps = _host_prep(positions, hidden_states, wqkv, wo, q_norm_w, k_norm_w)
    res = _run(in_maps, T, trace=_trace)
    NTb, NOBp = T // TB, HID // 256
    acc = np.zeros((NTb, NOBp, 128, 2, TB), np.float64)
    for r in res.results:
        acc += r["outT"].astype(np.float64).reshape(NTb, NOBp, 128, 2, TB)
    # untile: out[t, o] with o = (2*obp + oi)*128 + p, t = tb*TB + u
    out = np.ascontiguousarray(
        acc.transpose(0, 4, 1, 3, 2).reshape(T, HID)
    ).astype(np.float32)
    kernel._last_results = res
    return out
